# revision 34
# baseline (speedup 1.0000x reference)
"""Griffin block (Hawk RG-LRU + GatedMLP) Trainium2 Bass kernel, v4.

Sharding: 8 chunks = 4 batches x 2 time-halves, one per NeuronCore.
Per-core layout is feature-major ([channels, tokens]); everything bf16
except the recurrence coefficients (alpha stays f32) and the final
residual adds (psum f32 + bf16 carrier).

v4 notes:
  - sigmoids replaced with tanh (sigma(x) = (1+tanh(x/2))/2 folded into
    downstream scales/biases) so the forget/input gates, alpha and a2
    exps all live in ONE activation-table set (exp_and_others has both
    exp and tanh); Ln resolves to natural_log_exp_and_others via the
    table patch in this file, so the beta chain stays resident too.
  - per-(tensor, tile) wide SBUF tiles with a single batched DMA each
    (a dma_start costs ~0.6us of engine time; v3 issued ~450 of them).
  - phase 1 runs a two-tile software pipeline; hawk gate projection,
    gelu and the carry AllGather fill the scan tail (transition);
    phase 2a (output proj + residual + norm2) then 2b (grow/shrink).
"""

import numpy as np
import ml_dtypes
from contextlib import ExitStack

import concourse.bass as bass
import concourse.bacc as bacc
import concourse.tile as tile
from concourse import mybir
from concourse.bass_utils import run_bass_kernel_spmd

# The act-table-load pass maps each activation to the FIRST table set that
# contains it: Ln -> "natural_log" (ln only), Exp -> "exp_and_others", which
# thrashes a table load on every Ln<->Exp alternation.  Dropping `ln` from
# the ln-only set makes Ln resolve to "natural_log_exp_and_others" (has BOTH
# ln and exp), so ln/exp stretches share one resident set.  Set ids keep
# their act_info.json positions; the hardware tables are unchanged.
import concourse.hw_specs as _hw_specs


def _patched_act_tables(arch):
    tabs = _hw_specs.get_activation_tables(arch)
    out = {}
    for name, fns in tabs.items():
        if name == "natural_log":
            fns = fns - {mybir.ActivationFunctionType.Ln}
        out[name] = fns
    return out


bacc.get_activation_tables = _patched_act_tables
import concourse.bass_interp as _bass_interp
_bass_interp.get_activation_tables = _patched_act_tables

F32 = mybir.dt.float32
BF16 = mybir.dt.bfloat16
AF = mybir.ActivationFunctionType
OP = mybir.AluOpType

D = 1024
NP = 128          # partitions
NCT = D // NP     # channel tiles = 8
KCONV = 4
N_CORES = 8

_BF = ml_dtypes.bfloat16


def build_program(T_core: int, L: int, gelu_approx: bool = False):
    """Emit the SPMD program. T_core tokens per core, token tile L."""
    assert T_core % L == 0
    n_tiles = T_core // L
    H2 = 2 * D        # hawk proj width (2048)
    HID = 2 * H2      # gmlp hidden rows (4096): gate2 [0:2048), v [2048:4096)
    NGT = H2 // NP    # 16 gate/v ctiles
    W = NCT * L       # wide tile free size

    nc = bacc.Bacc("TRN2", target_bir_lowering=False, debug=False,
                   num_devices=N_CORES)

    # ---- DRAM parameters (per-core data via in_maps) ----
    x_d = nc.dram_tensor("x", [D, 3 + T_core], BF16, kind="ExternalInput")
    wiz_d = nc.dram_tensor("wiz", [D, D], BF16, kind="ExternalInput")    # z rows of input_w.T (gamma folded)
    wig_d = nc.dram_tensor("wig", [D, D], BF16, kind="ExternalInput")    # gate rows
    wg_d = nc.dram_tensor("wg", [D, H2], BF16, kind="ExternalInput")     # gates_w.T
    wo_d = nc.dram_tensor("wo", [D, D], BF16, kind="ExternalInput")      # output_w.T
    wgr_d = nc.dram_tensor("wgr", [D, HID], BF16, kind="ExternalInput")  # grow_w.T (gamma folded)
    wsh_d = nc.dram_tensor("wsh", [H2, D], BF16, kind="ExternalInput")   # shrink_w.T
    # per-channel params, laid out [partition, ch_tile]
    msp_d = nc.dram_tensor("msp", [NP, NCT], F32, kind="ExternalInput")    # -8*softplus(fb)
    msph_d = nc.dram_tensor("msph", [NP, NCT], F32, kind="ExternalInput")  # msp/2
    gbfh_d = nc.dram_tensor("gbfh", [NP, NCT], F32, kind="ExternalInput")  # gates_b[:D]/2
    gbih_d = nc.dram_tensor("gbih", [NP, NCT], F32, kind="ExternalInput")  # gates_b[D:]/2
    cw_d = nc.dram_tensor("cw", [NP, KCONV * NCT], F32, kind="ExternalInput")  # conv w taps
    cb_d = nc.dram_tensor("cb", [NP, NCT], F32, kind="ExternalInput")      # conv bias
    cmask_d = nc.dram_tensor("cmask", [NP, 1], F32, kind="ExternalInput")  # 1.0 iff second half

    out_d = nc.dram_tensor("out", [D, T_core], F32, kind="ExternalOutput")

    # ---- internal DRAM scratch ----
    h_d = nc.dram_tensor("h_spill", [D, T_core], BF16)
    ac_d = nc.dram_tensor("ac_spill", [D, T_core], BF16)
    x1_d = nc.dram_tensor("x1_spill", [D, T_core], BF16)
    g_d = nc.dram_tensor("g_spill", [D, T_core], BF16)
    carry_loc = nc.dram_tensor("carry_loc", [1, D], F32)
    carry_gth = nc.dram_tensor("carry_gth", [2, D], F32)

    def wide_in(dram, c0, w):
        """[D', c0:c0+w] -> [128, (D'/128)*w] AP (ctile-major free dim)."""
        return dram.ap()[:, c0:c0 + w].rearrange("(c p) t -> p c t", p=NP)

    with tile.TileContext(nc) as tc, ExitStack() as top:
        # ------- persistent small constants -------
        cpool = top.enter_context(tc.tile_pool(name="consts", bufs=1))
        ones_bf = cpool.tile([NP, NP], BF16, name="ones_bf")
        nc.vector.memset(ones_bf[:], 1.0)
        ones_f = cpool.tile([NP, L], F32, name="ones_f")
        nc.vector.memset(ones_f[:], 1.0)
        msp_sb = cpool.tile([NP, NCT], F32, name="msp_sb")
        nc.sync.dma_start(msp_sb[:], msp_d.ap()[:, :])
        msph_sb = cpool.tile([NP, NCT], F32, name="msph_sb")
        nc.sync.dma_start(msph_sb[:], msph_d.ap()[:, :])
        gbfh_sb = cpool.tile([NP, NCT], F32, name="gbfh_sb")
        nc.sync.dma_start(gbfh_sb[:], gbfh_d.ap()[:, :])
        gbih_sb = cpool.tile([NP, NCT], F32, name="gbih_sb")
        nc.sync.dma_start(gbih_sb[:], gbih_d.ap()[:, :])
        cw_sb = cpool.tile([NP, KCONV * NCT], F32, name="cw_sb")
        nc.sync.dma_start(cw_sb[:], cw_d.ap()[:, :])
        cb_sb = cpool.tile([NP, NCT], F32, name="cb_sb")
        nc.sync.dma_start(cb_sb[:], cb_d.ap()[:, :])
        cmask_sb = cpool.tile([NP, 1], F32, name="cmask_sb")
        nc.sync.dma_start(cmask_sb[:], cmask_d.ap()[:, :])
        hlast = cpool.tile([NP, NCT], F32, name="hlast")
        alast = cpool.tile([NP, NCT], F32, name="alast")
        epsb = cpool.tile([NP, 1], F32, name="epsb")
        nc.vector.memset(epsb[:], 1e-20)
        onepb = cpool.tile([NP, 1], F32, name="onepb")
        nc.vector.memset(onepb[:], 1.0 + 1e-6)
        lhalfb = cpool.tile([NP, 1], F32, name="lhalfb")
        nc.vector.memset(lhalfb[:], float(np.log(0.5)))

        def emit_gelu(out_ap, ps, pool, tag):
            """gelu(ps) -> out; sim lacks Gelu so approx mode builds it."""
            if gelu_approx:
                sg = pool.tile([NP, ps.shape[-1]], F32, name=f"sg_{tag}",
                               tag="gelu_sg")
                nc.scalar.activation(sg[:], ps, AF.Sigmoid, scale=1.702)
                nc.vector.tensor_tensor(out_ap, ps, sg[:], OP.mult)
            else:
                nc.scalar.activation(out_ap, ps, AF.Gelu)

        # xn persists through the transition (gate projs consume it)
        xn_scope = ExitStack()
        xnp = xn_scope.enter_context(
            tc.tile_pool(name="xnp", bufs=NCT * n_tiles))
        xn_t = [None] * n_tiles          # [t][i] -> [NP, L] bf16

        # =========================== PHASE 1 ===========================
        p1 = ExitStack()
        wpool = p1.enter_context(tc.tile_pool(name="w1", bufs=1))
        wiz_sb = wpool.tile([NP, NCT * D], BF16, name="wiz_sb")
        wg_sb = wpool.tile([NP, NCT * H2], BF16, name="wg_sb")

        xp = p1.enter_context(tc.tile_pool(name="xp", bufs=3))
        hxp = p1.enter_context(tc.tile_pool(name="hxp", bufs=NCT + 1))
        sbp = p1.enter_context(tc.tile_pool(name="sbp", bufs=3))   # bf16 temps
        sfp32 = p1.enter_context(tc.tile_pool(name="sfp32", bufs=2))  # f32 temps
        zp = p1.enter_context(tc.tile_pool(name="zp", bufs=NCT + 2))
        zc0p = p1.enter_context(tc.tile_pool(name="zc0p", bufs=4))
        zcbp = p1.enter_context(tc.tile_pool(name="zcbp", bufs=3 * NCT))
        sfp = p1.enter_context(tc.tile_pool(name="sfp", bufs=4))
        ap_ = p1.enter_context(tc.tile_pool(name="ap", bufs=3))
        bp_ = p1.enter_context(tc.tile_pool(name="bp", bufs=2))    # f32 a2/lu2
        bp16 = p1.enter_context(tc.tile_pool(name="bp16", bufs=3))  # bf16 b/sz/xs
        hp = p1.enter_context(tc.tile_pool(name="hp", bufs=1))      # wide h/ac
        zhp = p1.enter_context(tc.tile_pool(name="zhp", bufs=2))
        pmm = p1.enter_context(tc.tile_pool(name="pmm", bufs=5, space="PSUM"))
        pssq = p1.enter_context(tc.tile_pool(name="pssq", bufs=2, space="PSUM"))

        def load_x(t):
            xt = xp.tile([NP, W], BF16, name=f"x_{t}", tag="x")
            nc.sync.dma_start(xt[:], wide_in(x_d, 3 + t * L, L))
            return xt

        def norm_tiles(x_w, w, tag, pool, xsl):
            """s = exp(-0.5*ln(ssq)) = 1/||x||; xn = x*s (bf16).
            x_w wide tile; xsl(i) -> slice of ctile i."""
            ssq = pssq.tile([NP, w], F32, name=f"ssq_{tag}", tag="ssq")
            for i in range(NCT):
                xsq = sbp.tile([NP, w], BF16, name=f"xsq_{tag}_{i}", tag="xsq")
                nc.scalar.activation(xsq[:], xsl(x_w, i), AF.Square)
                nc.tensor.matmul(ssq[:], ones_bf[:], xsq[:],
                                 start=(i == 0), stop=(i == NCT - 1))
            lssq = sfp32.tile([NP, w], F32, name=f"lssq_{tag}", tag="lssq")
            nc.scalar.activation(lssq[:], ssq[:], AF.Ln, bias=epsb[:, 0:1])
            s = sbp.tile([NP, w], BF16, name=f"s_{tag}", tag="s")
            nc.scalar.activation(s[:], lssq[:], AF.Exp, scale=-0.5)
            xn = []
            for i in range(NCT):
                t_ = pool.tile([NP, w], BF16, name=f"xn_{tag}_{i}", tag="xn")
                nc.vector.tensor_tensor(t_[:], xsl(x_w, i), s[:], OP.mult)
                xn.append(t_)
            return xn

        def xsl_L(x_w, i):
            return x_w[:, i * L:(i + 1) * L]

        # ---- halo z: conv inputs for the 3 tokens before this chunk ----
        xh = xp.tile([NP, 3 * NCT], BF16, name="xh", tag="xh")
        nc.sync.dma_start(xh[:], wide_in(x_d, 0, 3))
        nc.sync.dma_start(wiz_sb[:], wide_in(wiz_d, 0, D))
        nc.scalar.dma_start(wg_sb[:], wide_in(wg_d, 0, H2))
        xnh = norm_tiles(xh, 3, "h", hxp, lambda w_, i: w_[:, 3 * i:3 * i + 3])
        zhalo_prev = zhp.tile([NP, 3 * NCT], BF16, name="zhalo_h", tag="zhalo")
        for m in range(NCT):
            ps = pmm.tile([NP, 3], F32, name=f"zh_ps_{m}", tag="mm")
            for k in range(NCT):
                lhs = wiz_sb[:, k * D + m * NP: k * D + (m + 1) * NP]
                nc.tensor.matmul(ps[:], lhs, xnh[k][:],
                                 start=(k == 0), stop=(k == NCT - 1))
            nc.vector.tensor_copy(zhalo_prev[:, 3 * m:3 * m + 3], ps[:])

        x_t = [None] * n_tiles
        zcb_t = [None] * n_tiles

        def stage_AB(t):
            """norm + z-proj + conv for tile t."""
            nonlocal zhalo_prev
            xn_t[t] = norm_tiles(x_t[t], L, f"t{t}", xnp, xsl_L)
            zhalo_cur = zhp.tile([NP, 3 * NCT], BF16, name=f"zhalo_{t}",
                                 tag="zhalo")
            zcb = []
            for m in range(NCT):
                ps = pmm.tile([NP, L], F32, name=f"z_ps_{t}_{m}", tag="mm")
                for k in range(NCT):
                    lhs = wiz_sb[:, k * D + m * NP: k * D + (m + 1) * NP]
                    nc.tensor.matmul(ps[:], lhs, xn_t[t][k][:],
                                     start=(k == 0), stop=(k == NCT - 1))
                zt = zp.tile([NP, L + 3], BF16, name=f"z_{t}_{m}", tag="z")
                nc.gpsimd.tensor_copy(zt[:, 0:3], zhalo_prev[:, 3 * m:3 * m + 3])
                nc.vector.tensor_copy(zt[:, 3:3 + L], ps[:])
                nc.gpsimd.tensor_copy(zhalo_cur[:, 3 * m:3 * m + 3],
                                      zt[:, L:L + 3])
                # depthwise causal conv: tap0 via tensor_scalar (w + bias),
                # taps 1-3 via STT; all bf16
                zc0 = zc0p.tile([NP, L], BF16, name=f"zc_{t}_{m}", tag="zc")
                nc.vector.tensor_scalar(zc0[:], zt[:, 0:L],
                                        cw_sb[:, 0 * NCT + m:0 * NCT + m + 1],
                                        cb_sb[:, m:m + 1],
                                        op0=OP.mult, op1=OP.add)
                for k in (1, 2):
                    nc.vector.scalar_tensor_tensor(
                        zc0[:], zt[:, k:k + L],
                        cw_sb[:, k * NCT + m:k * NCT + m + 1],
                        zc0[:], op0=OP.mult, op1=OP.add)
                zcbm = zcbp.tile([NP, L], BF16, name=f"zcb_{t}_{m}", tag="zcb")
                nc.vector.scalar_tensor_tensor(
                    zcbm[:], zt[:, 3:3 + L],
                    cw_sb[:, 3 * NCT + m:3 * NCT + m + 1],
                    zc0[:], op0=OP.mult, op1=OP.add)
                zcb.append(zcbm)
            zhalo_prev = zhalo_cur
            zcb_t[t] = zcb

        def stage_C(t):
            """gates proj + tanh gates + alpha/beta + scans + spill, tile t.

            sigma(v) = (1+tanh(v/2))/2:
              tf = tanh(0.5*psf + gbf/2)      alpha = exp(msph*tf + msph)
              a2 = exp(msp*tf + msp)          beta' = exp(0.5*lu2 + ln 0.5)
              ti = tanh(0.5*psi + gbi/2)      xs = ((1+ti)*zcb) * beta'
            Scalar blocks are batched per table set (tanh/exp share one)."""
            zcb = zcb_t[t]
            tf = [None] * NCT
            ti = [None] * NCT
            for i in range(NCT):
                psf = pmm.tile([NP, L], F32, name=f"f_ps_{t}_{i}", tag="mm")
                for k in range(NCT):
                    lhs = wg_sb[:, k * H2 + i * NP: k * H2 + (i + 1) * NP]
                    nc.tensor.matmul(psf[:], lhs, zcb[k][:],
                                     start=(k == 0), stop=(k == NCT - 1))
                tfi = sfp.tile([NP, L], BF16, name=f"tf_{t}_{i}", tag="tf")
                nc.scalar.activation(tfi[:], psf[:], AF.Tanh, scale=0.5,
                                     bias=gbfh_sb[:, i:i + 1])
                tf[i] = tfi
                psi = pmm.tile([NP, L], F32, name=f"i_ps_{t}_{i}", tag="mm")
                for k in range(NCT):
                    lhs = wg_sb[:, k * H2 + D + i * NP: k * H2 + D + (i + 1) * NP]
                    nc.tensor.matmul(psi[:], lhs, zcb[k][:],
                                     start=(k == 0), stop=(k == NCT - 1))
                tii = sfp.tile([NP, L], BF16, name=f"ti_{t}_{i}", tag="ti")
                nc.scalar.activation(tii[:], psi[:], AF.Tanh, scale=0.5,
                                     bias=gbih_sb[:, i:i + 1])
                ti[i] = tii

            alpha = [None] * NCT
            a2 = [None] * NCT
            for i in range(NCT):
                al = ap_.tile([NP, L], F32, name=f"al_{t}_{i}", tag="alpha")
                nc.scalar.activation(al[:], tf[i][:], AF.Exp,
                                     scale=msph_sb[:, i:i + 1],
                                     bias=msph_sb[:, i:i + 1])
                alpha[i] = al
                a2i = bp_.tile([NP, L], F32, name=f"a2_{t}_{i}", tag="a2")
                nc.scalar.activation(a2i[:], tf[i][:], AF.Exp,
                                     scale=msp_sb[:, i:i + 1],
                                     bias=msp_sb[:, i:i + 1])
                a2[i] = a2i
            beta = [None] * NCT
            for i in range(NCT):
                lu2 = bp_.tile([NP, L], F32, name=f"lu2_{t}_{i}", tag="lu2")
                nc.scalar.activation(lu2[:], a2[i][:], AF.Ln, scale=-1.0,
                                     bias=onepb[:, 0:1])
                be = bp16.tile([NP, L], BF16, name=f"be_{t}_{i}", tag="beta")
                nc.scalar.activation(be[:], lu2[:], AF.Exp, scale=0.5,
                                     bias=lhalfb[:, 0:1])
                beta[i] = be

            h_w = hp.tile([NP, W], BF16, name=f"h_{t}", tag="h")
            ac_w = hp.tile([NP, W], BF16, name=f"ac_{t}", tag="ac")
            for i in range(NCT):
                sz = bp16.tile([NP, L], BF16, name=f"sz_{t}_{i}", tag="sz")
                nc.vector.scalar_tensor_tensor(sz[:], ti[i][:], 1.0,
                                               zcb[i][:],
                                               op0=OP.add, op1=OP.mult)
                xs = bp16.tile([NP, L], BF16, name=f"xs_{t}_{i}", tag="xs")
                nc.gpsimd.tensor_tensor(xs[:], sz[:], beta[i][:], OP.mult)

                hsl = h_w[:, i * L:(i + 1) * L]
                h_init = 0.0 if t == 0 else hlast[:, i:i + 1]
                nc.vector.tensor_tensor_scan(hsl, alpha[i][:], xs[:], h_init,
                                             op0=OP.mult, op1=OP.add)
                nc.gpsimd.tensor_copy(hlast[:, i:i + 1],
                                      h_w[:, (i + 1) * L - 1:(i + 1) * L])
                asl = ac_w[:, i * L:(i + 1) * L]
                a_init = 1.0 if t == 0 else alast[:, i:i + 1]
                nc.vector.tensor_tensor_scan(asl, alpha[i][:], ones_f[:, 0:L],
                                             a_init, op0=OP.mult, op1=OP.mult)
                nc.gpsimd.tensor_copy(alast[:, i:i + 1],
                                      ac_w[:, (i + 1) * L - 1:(i + 1) * L])
            nc.sync.dma_start(wide_in(h_d, t * L, L), h_w[:])
            nc.gpsimd.dma_start(wide_in(ac_d, t * L, L), ac_w[:])
            if t == n_tiles - 1:
                nc.scalar.dma_start(
                    carry_loc.ap()[0:1, :].rearrange("a (c p) -> p (a c)",
                                                     p=NP),
                    hlast[:])

        # -------- pipelined emission: A/B two tiles ahead of C --------
        x_t[0] = load_x(0)
        if n_tiles > 1:
            x_t[1] = load_x(1)
        stage_AB(0)
        if n_tiles > 1:
            stage_AB(1)
        for t in range(n_tiles):
            if t + 2 < n_tiles:
                x_t[t + 2] = load_x(t + 2)
                stage_AB(t + 2)
            stage_C(t)
            x_t[t] = None  # allow pool reuse

        p1.close()

        # ================= TRANSITION: gate projs + carry =================
        tr = ExitStack()
        wpool_g = tr.enter_context(tc.tile_pool(name="wg2", bufs=1))
        wig_sb = wpool_g.tile([NP, NCT * D], BF16, name="wig_sb")
        nc.sync.dma_start(wig_sb[:], wide_in(wig_d, 0, D))
        pmg = tr.enter_context(tc.tile_pool(name="pmg", bufs=4, space="PSUM"))
        gpool = tr.enter_context(tc.tile_pool(name="gpool", bufs=2))

        # pairwise carry exchange first: gate projs fill the PE while the
        # collective runs; carry reads ride the vector queue so the sync
        # queue's phase-2a reload DMAs are not head-of-line blocked.
        nc.gpsimd.collective_compute(
            "AllGather", OP.bypass,
            replica_groups=[[0, 1], [2, 3], [4, 5], [6, 7]],
            ins=[carry_loc.ap()], outs=[carry_gth.ap()])
        cg = cpool.tile([NP, NCT], F32, name="cg")
        nc.scalar.dma_start(
            cg[:],
            carry_gth.ap()[0:1, :].rearrange("a (c p) -> p (a c)", p=NP))
        carrym = cpool.tile([NP, NCT], F32, name="carrym")
        nc.vector.tensor_scalar(carrym[:], cg[:], cmask_sb[:, 0:1], None,
                                op0=OP.mult)

        for t in range(n_tiles):
            g_w = gpool.tile([NP, W], BF16, name=f"g_{t}", tag="g")
            for m in range(NCT):
                ps = pmg.tile([NP, L], F32, name=f"g_ps_{t}_{m}", tag="mmg")
                for k in range(NCT):
                    lhs = wig_sb[:, k * D + m * NP: k * D + (m + 1) * NP]
                    nc.tensor.matmul(ps[:], lhs, xn_t[t][k][:],
                                     start=(k == 0), stop=(k == NCT - 1))
                emit_gelu(g_w[:, m * L:(m + 1) * L], ps[:], gpool, f"g_{t}_{m}")
            nc.sync.dma_start(wide_in(g_d, t * L, L), g_w[:])
        tr.close()
        xn_scope.close()

        # =========================== PHASE 2a ===========================
        # long-lived pools first (outlive p2a; LIFO discipline)
        wpool_gr = top.enter_context(tc.tile_pool(name="wgr2", bufs=1))
        wgr_sb = wpool_gr.tile([NP, NCT * HID], BF16, name="wgr_sb")
        x1np = top.enter_context(tc.tile_pool(name="x1np",
                                              bufs=NCT * n_tiles))
        p2a = ExitStack()
        wpool_o = p2a.enter_context(tc.tile_pool(name="wo2", bufs=1))
        wo_sb = wpool_o.tile([NP, NCT * D], BF16, name="wo_sb")
        nc.sync.dma_start(wo_sb[:], wide_in(wo_d, 0, D))
        # grow weights start loading now (consumed in 2b)
        nc.scalar.dma_start(wgr_sb[:], wide_in(wgr_d, 0, HID))

        hp2 = p2a.enter_context(tc.tile_pool(name="hp2", bufs=1))
        grp = p2a.enter_context(tc.tile_pool(name="grp", bufs=1))
        xp2 = p2a.enter_context(tc.tile_pool(name="xp2", bufs=1))
        sp2b = p2a.enter_context(tc.tile_pool(name="sp2b", bufs=4))   # bf16
        sp2f = p2a.enter_context(tc.tile_pool(name="sp2f", bufs=1))   # f32
        ghp = p2a.enter_context(tc.tile_pool(name="ghp", bufs=NCT + 2))
        x1bp = p2a.enter_context(tc.tile_pool(name="x1bp", bufs=2))
        pmm2 = p2a.enter_context(tc.tile_pool(name="pmm2", bufs=4, space="PSUM"))
        pssq2 = p2a.enter_context(tc.tile_pool(name="pssq2", bufs=2,
                                               space="PSUM"))
        x1n_t = [[None] * NCT for _ in range(n_tiles)]

        for t in range(n_tiles):
            hr_w = hp2.tile([NP, W], BF16, name=f"hr_{t}", tag="hr")
            nc.sync.dma_start(hr_w[:], wide_in(h_d, t * L, L))
            ac_w = hp2.tile([NP, W], BF16, name=f"acr_{t}", tag="acr")
            nc.sync.dma_start(ac_w[:], wide_in(ac_d, t * L, L))
            gr_w = grp.tile([NP, W], BF16, name=f"grl_{t}", tag="grl")
            nc.sync.dma_start(gr_w[:], wide_in(g_d, t * L, L))
            x2_w = xp2.tile([NP, W], BF16, name=f"x2_{t}", tag="x2")
            nc.sync.dma_start(x2_w[:], wide_in(x_d, 3 + t * L, L))

            gh = []
            for i in range(NCT):
                hf = sp2b.tile([NP, L], BF16, name=f"hf_{t}_{i}", tag="hf")
                nc.vector.scalar_tensor_tensor(
                    hf[:], ac_w[:, i * L:(i + 1) * L], carrym[:, i:i + 1],
                    hr_w[:, i * L:(i + 1) * L], op0=OP.mult, op1=OP.add)
                ghi = ghp.tile([NP, L], BF16, name=f"gh_{t}_{i}", tag="gh")
                nc.vector.tensor_tensor(ghi[:], gr_w[:, i * L:(i + 1) * L],
                                        hf[:], OP.mult)
                gh.append(ghi)

            # output proj + residual -> x1 (bf16, wide) -> spill to DRAM
            x1_w = x1bp.tile([NP, W], BF16, name=f"x1_{t}", tag="x1")
            for m in range(NCT):
                ps = pmm2.tile([NP, L], F32, name=f"o_ps_{t}_{m}", tag="mm2")
                for k in range(NCT):
                    lhs = wo_sb[:, k * D + m * NP: k * D + (m + 1) * NP]
                    nc.tensor.matmul(ps[:], lhs, gh[k][:],
                                     start=(k == 0), stop=(k == NCT - 1))
                nc.vector.tensor_tensor(x1_w[:, m * L:(m + 1) * L], ps[:],
                                        x2_w[:, m * L:(m + 1) * L], OP.add)
            nc.sync.dma_start(wide_in(x1_d, t * L, L), x1_w[:])

            # rmsnorm2 -> x1n (bf16); squares on scalar (in-set everywhere)
            ssq = pssq2.tile([NP, L], F32, name=f"ssq2_{t}", tag="ssq2")
            for i in range(NCT):
                xsq = sp2b.tile([NP, L], BF16, name=f"x1sq_{t}_{i}", tag="x1sq")
                nc.scalar.activation(xsq[:], x1_w[:, i * L:(i + 1) * L],
                                     AF.Square)
                nc.tensor.matmul(ssq[:], ones_bf[:], xsq[:],
                                 start=(i == 0), stop=(i == NCT - 1))
            lssq = sp2f.tile([NP, L], F32, name=f"lssq2_{t}", tag="lssq2")
            nc.scalar.activation(lssq[:], ssq[:], AF.Ln, bias=epsb[:, 0:1])
            s2 = sp2b.tile([NP, L], BF16, name=f"s2_{t}", tag="s2")
            nc.scalar.activation(s2[:], lssq[:], AF.Exp, scale=-0.5)
            for i in range(NCT):
                xni = x1np.tile([NP, L], BF16, name=f"x1n_{t}_{i}", tag="x1n")
                nc.vector.tensor_tensor(xni[:], x1_w[:, i * L:(i + 1) * L],
                                        s2[:], OP.mult)
                x1n_t[t][i] = xni

        p2a.close()

        # =========================== PHASE 2b ===========================
        p2b = ExitStack()
        wpool_s = p2b.enter_context(tc.tile_pool(name="ws2", bufs=1))
        wsh_sb = wpool_s.tile([NP, 2 * NCT * D], BF16, name="wsh_sb")
        nc.sync.dma_start(wsh_sb[:], wide_in(wsh_d, 0, D))

        gvp = p2b.enter_context(tc.tile_pool(name="gvp", bufs=2 * NGT + 2))
        t2p = p2b.enter_context(tc.tile_pool(name="t2p", bufs=4))
        x1rp = p2b.enter_context(tc.tile_pool(name="x1rp", bufs=2))
        op_ = p2b.enter_context(tc.tile_pool(name="op", bufs=6))
        pmmg = p2b.enter_context(tc.tile_pool(name="pmmg", bufs=4, space="PSUM"))
        pmms = p2b.enter_context(tc.tile_pool(name="pmms", bufs=3, space="PSUM"))

        gv_t = [[None] * NGT for _ in range(n_tiles)]
        x1r_t = [None] * n_tiles

        def stage_grow(t):
            # prefetch x1 reload for the shrink residual
            x1r = x1rp.tile([NP, W], BF16, name=f"x1r_{t}", tag="x1r")
            nc.gpsimd.dma_start(x1r[:], wide_in(x1_d, t * L, L))
            x1r_t[t] = x1r
            for hm in range(NGT):
                psg = pmmg.tile([NP, L], F32, name=f"g2_ps_{t}_{hm}", tag="mmg2")
                for k in range(NCT):
                    lhs = wgr_sb[:, k * HID + hm * NP: k * HID + (hm + 1) * NP]
                    nc.tensor.matmul(psg[:], lhs, x1n_t[t][k][:],
                                     start=(k == 0), stop=(k == NCT - 1))
                psv = pmmg.tile([NP, L], F32, name=f"v_ps_{t}_{hm}", tag="mmg2")
                for k in range(NCT):
                    lhs = wgr_sb[:, k * HID + H2 + hm * NP:
                                 k * HID + H2 + (hm + 1) * NP]
                    nc.tensor.matmul(psv[:], lhs, x1n_t[t][k][:],
                                     start=(k == 0), stop=(k == NCT - 1))
                t2 = t2p.tile([NP, L], BF16, name=f"t2_{t}_{hm}", tag="t2")
                emit_gelu(t2[:], psg[:], t2p, f"t2_{t}_{hm}")
                gvi = gvp.tile([NP, L], BF16, name=f"gv_{t}_{hm}", tag="gv")
                nc.vector.tensor_tensor(gvi[:], t2[:], psv[:], OP.mult)
                gv_t[t][hm] = gvi

        def stage_shrink(t):
            for m in range(NCT):
                ps = pmms.tile([NP, L], F32, name=f"s_ps_{t}_{m}", tag="mms")
                for k in range(2 * NCT):
                    lhs = wsh_sb[:, k * D + m * NP: k * D + (m + 1) * NP]
                    nc.tensor.matmul(ps[:], lhs, gv_t[t][k][:],
                                     start=(k == 0), stop=(k == 2 * NCT - 1))
                om = op_.tile([NP, L], F32, name=f"out_{t}_{m}", tag="out")
                nc.vector.tensor_tensor(om[:], ps[:],
                                        x1r_t[t][:, m * L:(m + 1) * L], OP.add)
                nc.sync.dma_start(
                    out_d.ap()[m * NP:(m + 1) * NP, t * L:(t + 1) * L], om[:])

        stage_grow(0)
        for t in range(1, n_tiles):
            stage_grow(t)
            stage_shrink(t - 1)
        stage_shrink(n_tiles - 1)
        p2b.close()

    nc.compile()
    return nc


def host_prepare(inputs, T_core, n_cores=N_CORES):
    """Build per-core in_maps from full inputs."""
    x = np.asarray(inputs["x"], np.float32)            # [B, T, D]
    B, T, _ = x.shape
    halves = n_cores // B
    assert T == halves * T_core

    gam1 = np.asarray(inputs["hawk_norm_gamma"], np.float32)
    gam2 = np.asarray(inputs["gmlp_norm_gamma"], np.float32)
    scale1 = gam1 * np.sqrt(D)
    scale2 = gam2 * np.sqrt(D)

    wi = (np.asarray(inputs["input_w"], np.float32) * scale1[None, :]).T
    wg = np.asarray(inputs["gates_w"], np.float32).T
    wo = np.asarray(inputs["output_w"], np.float32).T
    wgr = (np.asarray(inputs["grow_w"], np.float32) * scale2[None, :]).T
    wsh = np.asarray(inputs["shrink_w"], np.float32).T

    fb = np.asarray(inputs["forget_base"], np.float64)
    msp = (-8.0 * np.log1p(np.exp(fb))).astype(np.float32)

    def chan_layout(v):  # [D] -> [128, 8] with [p, i] = v[128*i + p]
        return np.ascontiguousarray(v.reshape(NCT, NP).T)

    gb = np.asarray(inputs["gates_b"], np.float32)
    cw = np.asarray(inputs["conv_w"], np.float32)[:, 0, :]   # [D, K]
    cb = np.asarray(inputs["conv_b"], np.float32)

    shared = {
        "wig": np.ascontiguousarray(wi[:, :D]).astype(_BF),
        "wiz": np.ascontiguousarray(wi[:, D:]).astype(_BF),
        "wg": wg.astype(_BF), "wo": wo.astype(_BF),
        "wgr": wgr.astype(_BF), "wsh": wsh.astype(_BF),
        "msp": chan_layout(msp), "msph": chan_layout(0.5 * msp),
        "gbfh": chan_layout(0.5 * gb[:D]), "gbih": chan_layout(0.5 * gb[D:]),
        "cw": np.concatenate([chan_layout(cw[:, k]) for k in range(KCONV)],
                             axis=1),
        "cb": chan_layout(cb),
    }
    in_maps = []
    for core in range(n_cores):
        b, h = core // halves, core % halves
        xf = np.zeros((D, 3 + T_core), np.float32)
        xf[:, 3:] = x[b, h * T_core:(h + 1) * T_core, :].T
        if h > 0:
            xf[:, 0:3] = x[b, h * T_core - 3:h * T_core, :].T
        m = dict(shared)
        m["x"] = xf.astype(_BF)
        m["cmask"] = np.full((NP, 1), 1.0 if h > 0 else 0.0, np.float32)
        in_maps.append(m)
    return in_maps


def assemble_output(results, B, T, T_core, n_cores=N_CORES):
    halves = n_cores // B
    out = np.empty((B, T, D), np.float32)
    for core in range(n_cores):
        b, h = core // halves, core % halves
        out[b, h * T_core:(h + 1) * T_core, :] = results[core]["out"].T
    return out


_PROG_CACHE = {}


def kernel(**inputs) -> np.ndarray:
    x = np.asarray(inputs["x"])
    B, T, _ = x.shape
    T_core = T * B // N_CORES
    L = 512 if T_core % 512 == 0 else T_core // 4
    key = (T_core, L)
    if key not in _PROG_CACHE:
        _PROG_CACHE[key] = build_program(T_core, L)
    nc = _PROG_CACHE[key]
    in_maps = host_prepare(inputs, T_core)
    res = run_bass_kernel_spmd(nc, in_maps, list(range(N_CORES)))
    return assemble_output(res.results, B, T, T_core)


# revision 35
# speedup vs baseline: 1.0041x; 1.0041x over previous
"""Griffin block (Hawk RG-LRU + GatedMLP) Trainium2 Bass kernel, v4.

Sharding: 8 chunks = 4 batches x 2 time-halves, one per NeuronCore.
Per-core layout is feature-major ([channels, tokens]); everything bf16
except the recurrence coefficients (alpha stays f32) and the final
residual adds (psum f32 + bf16 carrier).

v4 notes:
  - sigmoids replaced with tanh (sigma(x) = (1+tanh(x/2))/2 folded into
    downstream scales/biases) so the forget/input gates, alpha and a2
    exps all live in ONE activation-table set (exp_and_others has both
    exp and tanh); Ln resolves to natural_log_exp_and_others via the
    table patch in this file, so the beta chain stays resident too.
  - per-(tensor, tile) wide SBUF tiles with a single batched DMA each
    (a dma_start costs ~0.6us of engine time; v3 issued ~450 of them).
  - phase 1 runs a two-tile software pipeline; hawk gate projection,
    gelu and the carry AllGather fill the scan tail (transition);
    phase 2a (output proj + residual + norm2) then 2b (grow/shrink).
"""

import numpy as np
import ml_dtypes
from contextlib import ExitStack

import concourse.bass as bass
import concourse.bacc as bacc
import concourse.tile as tile
from concourse import mybir
from concourse.bass_utils import run_bass_kernel_spmd

# The act-table-load pass maps each activation to the FIRST table set that
# contains it: Ln -> "natural_log" (ln only), Exp -> "exp_and_others", which
# thrashes a table load on every Ln<->Exp alternation.  Dropping `ln` from
# the ln-only set makes Ln resolve to "natural_log_exp_and_others" (has BOTH
# ln and exp), so ln/exp stretches share one resident set.  Set ids keep
# their act_info.json positions; the hardware tables are unchanged.
import concourse.hw_specs as _hw_specs


def _patched_act_tables(arch):
    tabs = _hw_specs.get_activation_tables(arch)
    out = {}
    for name, fns in tabs.items():
        if name == "natural_log":
            fns = fns - {mybir.ActivationFunctionType.Ln}
        out[name] = fns
    return out


bacc.get_activation_tables = _patched_act_tables
import concourse.bass_interp as _bass_interp
_bass_interp.get_activation_tables = _patched_act_tables

F32 = mybir.dt.float32
BF16 = mybir.dt.bfloat16
AF = mybir.ActivationFunctionType
OP = mybir.AluOpType

D = 1024
NP = 128          # partitions
NCT = D // NP     # channel tiles = 8
KCONV = 4
N_CORES = 8

_BF = ml_dtypes.bfloat16


def build_program(T_core: int, L: int, gelu_approx: bool = False):
    """Emit the SPMD program. T_core tokens per core, token tile L."""
    assert T_core % L == 0
    n_tiles = T_core // L
    H2 = 2 * D        # hawk proj width (2048)
    HID = 2 * H2      # gmlp hidden rows (4096): gate2 [0:2048), v [2048:4096)
    NGT = H2 // NP    # 16 gate/v ctiles
    W = NCT * L       # wide tile free size

    nc = bacc.Bacc("TRN2", target_bir_lowering=False, debug=False,
                   num_devices=N_CORES)

    # ---- DRAM parameters (per-core data via in_maps) ----
    x_d = nc.dram_tensor("x", [D, 3 + T_core], BF16, kind="ExternalInput")
    wiz_d = nc.dram_tensor("wiz", [D, D], BF16, kind="ExternalInput")    # z rows of input_w.T (gamma folded)
    wig_d = nc.dram_tensor("wig", [D, D], BF16, kind="ExternalInput")    # gate rows
    wg_d = nc.dram_tensor("wg", [D, H2], BF16, kind="ExternalInput")     # gates_w.T
    wo_d = nc.dram_tensor("wo", [D, D], BF16, kind="ExternalInput")      # output_w.T
    wgr_d = nc.dram_tensor("wgr", [D, HID], BF16, kind="ExternalInput")  # grow_w.T (gamma folded)
    wsh_d = nc.dram_tensor("wsh", [H2, D], BF16, kind="ExternalInput")   # shrink_w.T
    # per-channel params, laid out [partition, ch_tile]
    msp_d = nc.dram_tensor("msp", [NP, NCT], F32, kind="ExternalInput")    # -8*softplus(fb)
    msph_d = nc.dram_tensor("msph", [NP, NCT], F32, kind="ExternalInput")  # msp/2
    gbfh_d = nc.dram_tensor("gbfh", [NP, NCT], F32, kind="ExternalInput")  # gates_b[:D]/2
    gbih_d = nc.dram_tensor("gbih", [NP, NCT], F32, kind="ExternalInput")  # gates_b[D:]/2
    cw_d = nc.dram_tensor("cw", [NP, KCONV * NCT], F32, kind="ExternalInput")  # conv w taps
    cb_d = nc.dram_tensor("cb", [NP, NCT], F32, kind="ExternalInput")      # conv bias
    cmask_d = nc.dram_tensor("cmask", [NP, 1], F32, kind="ExternalInput")  # 1.0 iff second half

    out_d = nc.dram_tensor("out", [D, T_core], F32, kind="ExternalOutput")

    # ---- internal DRAM scratch ----
    h_d = nc.dram_tensor("h_spill", [D, T_core], BF16)
    ac_d = nc.dram_tensor("ac_spill", [D, T_core], BF16)
    x1_d = nc.dram_tensor("x1_spill", [D, T_core], BF16)
    g_d = nc.dram_tensor("g_spill", [D, T_core], BF16)
    carry_loc = nc.dram_tensor("carry_loc", [1, D], F32)
    carry_gth = nc.dram_tensor("carry_gth", [2, D], F32)

    def wide_in(dram, c0, w):
        """[D', c0:c0+w] -> [128, (D'/128)*w] AP (ctile-major free dim)."""
        return dram.ap()[:, c0:c0 + w].rearrange("(c p) t -> p c t", p=NP)

    with tile.TileContext(nc) as tc, ExitStack() as top:
        # ------- persistent small constants -------
        cpool = top.enter_context(tc.tile_pool(name="consts", bufs=1))
        ones_bf = cpool.tile([NP, NP], BF16, name="ones_bf")
        nc.vector.memset(ones_bf[:], 1.0)
        ones_f = cpool.tile([NP, L], F32, name="ones_f")
        nc.vector.memset(ones_f[:], 1.0)
        msp_sb = cpool.tile([NP, NCT], F32, name="msp_sb")
        nc.sync.dma_start(msp_sb[:], msp_d.ap()[:, :])
        msph_sb = cpool.tile([NP, NCT], F32, name="msph_sb")
        nc.sync.dma_start(msph_sb[:], msph_d.ap()[:, :])
        gbfh_sb = cpool.tile([NP, NCT], F32, name="gbfh_sb")
        nc.sync.dma_start(gbfh_sb[:], gbfh_d.ap()[:, :])
        gbih_sb = cpool.tile([NP, NCT], F32, name="gbih_sb")
        nc.sync.dma_start(gbih_sb[:], gbih_d.ap()[:, :])
        cw_sb = cpool.tile([NP, KCONV * NCT], F32, name="cw_sb")
        nc.sync.dma_start(cw_sb[:], cw_d.ap()[:, :])
        cb_sb = cpool.tile([NP, NCT], F32, name="cb_sb")
        nc.sync.dma_start(cb_sb[:], cb_d.ap()[:, :])
        cmask_sb = cpool.tile([NP, 1], F32, name="cmask_sb")
        nc.sync.dma_start(cmask_sb[:], cmask_d.ap()[:, :])
        hlast = cpool.tile([NP, NCT], F32, name="hlast")
        alast = cpool.tile([NP, NCT], F32, name="alast")
        epsb = cpool.tile([NP, 1], F32, name="epsb")
        nc.vector.memset(epsb[:], 1e-20)
        onepb = cpool.tile([NP, 1], F32, name="onepb")
        nc.vector.memset(onepb[:], 1.0 + 1e-6)
        lhalfb = cpool.tile([NP, 1], F32, name="lhalfb")
        nc.vector.memset(lhalfb[:], float(np.log(0.5)))

        def emit_gelu(out_ap, ps, pool, tag):
            """gelu(ps) -> out; sim lacks Gelu so approx mode builds it."""
            if gelu_approx:
                sg = pool.tile([NP, ps.shape[-1]], F32, name=f"sg_{tag}",
                               tag="gelu_sg")
                nc.scalar.activation(sg[:], ps, AF.Sigmoid, scale=1.702)
                nc.vector.tensor_tensor(out_ap, ps, sg[:], OP.mult)
            else:
                nc.scalar.activation(out_ap, ps, AF.Gelu)

        # xn persists through the transition (gate projs consume it)
        xn_scope = ExitStack()
        xnp = xn_scope.enter_context(
            tc.tile_pool(name="xnp", bufs=NCT * n_tiles))
        xn_t = [None] * n_tiles          # [t][i] -> [NP, L] bf16

        # =========================== PHASE 1 ===========================
        p1 = ExitStack()
        wpool = p1.enter_context(tc.tile_pool(name="w1", bufs=1))
        wiz_sb = wpool.tile([NP, NCT * D], BF16, name="wiz_sb")
        wg_sb = wpool.tile([NP, NCT * H2], BF16, name="wg_sb")

        xp = p1.enter_context(tc.tile_pool(name="xp", bufs=3))
        hxp = p1.enter_context(tc.tile_pool(name="hxp", bufs=NCT + 1))
        sbp = p1.enter_context(tc.tile_pool(name="sbp", bufs=3))   # bf16 temps
        sfp32 = p1.enter_context(tc.tile_pool(name="sfp32", bufs=2))  # f32 temps
        zp = p1.enter_context(tc.tile_pool(name="zp", bufs=NCT + 2))
        zc0p = p1.enter_context(tc.tile_pool(name="zc0p", bufs=4))
        zcbp = p1.enter_context(tc.tile_pool(name="zcbp", bufs=3 * NCT))
        sfp = p1.enter_context(tc.tile_pool(name="sfp", bufs=4))
        ap_ = p1.enter_context(tc.tile_pool(name="ap", bufs=3))
        bp_ = p1.enter_context(tc.tile_pool(name="bp", bufs=2))    # f32 a2/lu2
        bp16 = p1.enter_context(tc.tile_pool(name="bp16", bufs=3))  # bf16 b/sz/xs
        hp = p1.enter_context(tc.tile_pool(name="hp", bufs=1))      # wide h/ac
        zhp = p1.enter_context(tc.tile_pool(name="zhp", bufs=2))
        pmm = p1.enter_context(tc.tile_pool(name="pmm", bufs=6, space="PSUM"))
        pssq = p1.enter_context(tc.tile_pool(name="pssq", bufs=2, space="PSUM"))

        def load_x(t):
            xt = xp.tile([NP, W], BF16, name=f"x_{t}", tag="x")
            hc = NCT // 2
            c0 = 3 + t * L
            nc.sync.dma_start(
                xt[:, 0:hc * L].rearrange("p (c t) -> p c t", c=hc),
                x_d.ap()[0:hc * NP, c0:c0 + L]
                .rearrange("(c p) t -> p c t", p=NP))
            nc.scalar.dma_start(
                xt[:, hc * L:].rearrange("p (c t) -> p c t", c=hc),
                x_d.ap()[hc * NP:D, c0:c0 + L]
                .rearrange("(c p) t -> p c t", p=NP))
            return xt

        def norm_tiles(x_w, w, tag, pool, xsl):
            """s = exp(-0.5*ln(ssq)) = 1/||x||; xn = x*s (bf16).
            x_w wide tile; xsl(i) -> slice of ctile i."""
            ssq = pssq.tile([NP, w], F32, name=f"ssq_{tag}", tag="ssq")
            for i in range(NCT):
                xsq = sbp.tile([NP, w], BF16, name=f"xsq_{tag}_{i}", tag="xsq")
                nc.scalar.activation(xsq[:], xsl(x_w, i), AF.Square)
                nc.tensor.matmul(ssq[:], ones_bf[:], xsq[:],
                                 start=(i == 0), stop=(i == NCT - 1))
            lssq = sfp32.tile([NP, w], F32, name=f"lssq_{tag}", tag="lssq")
            nc.scalar.activation(lssq[:], ssq[:], AF.Ln, bias=epsb[:, 0:1])
            s = sbp.tile([NP, w], BF16, name=f"s_{tag}", tag="s")
            nc.scalar.activation(s[:], lssq[:], AF.Exp, scale=-0.5)
            xn = []
            for i in range(NCT):
                t_ = pool.tile([NP, w], BF16, name=f"xn_{tag}_{i}", tag="xn")
                nc.vector.tensor_tensor(t_[:], xsl(x_w, i), s[:], OP.mult)
                xn.append(t_)
            return xn

        def xsl_L(x_w, i):
            return x_w[:, i * L:(i + 1) * L]

        # ---- halo z: conv inputs for the 3 tokens before this chunk ----
        xh = xp.tile([NP, 3 * NCT], BF16, name="xh", tag="xh")
        nc.sync.dma_start(xh[:], wide_in(x_d, 0, 3))
        nc.sync.dma_start(wiz_sb[:], wide_in(wiz_d, 0, D))
        nc.scalar.dma_start(wg_sb[:], wide_in(wg_d, 0, H2))
        xnh = norm_tiles(xh, 3, "h", hxp, lambda w_, i: w_[:, 3 * i:3 * i + 3])
        zhalo_prev = zhp.tile([NP, 3 * NCT], BF16, name="zhalo_h", tag="zhalo")
        for m in range(NCT):
            ps = pmm.tile([NP, 3], F32, name=f"zh_ps_{m}", tag="mm")
            for k in range(NCT):
                lhs = wiz_sb[:, k * D + m * NP: k * D + (m + 1) * NP]
                nc.tensor.matmul(ps[:], lhs, xnh[k][:],
                                 start=(k == 0), stop=(k == NCT - 1))
            nc.vector.tensor_copy(zhalo_prev[:, 3 * m:3 * m + 3], ps[:])

        x_t = [None] * n_tiles
        zcb_t = [None] * n_tiles

        def stage_AB(t):
            """norm + z-proj + conv for tile t."""
            nonlocal zhalo_prev
            xn_t[t] = norm_tiles(x_t[t], L, f"t{t}", xnp, xsl_L)
            zhalo_cur = zhp.tile([NP, 3 * NCT], BF16, name=f"zhalo_{t}",
                                 tag="zhalo")
            zcb = []
            for m in range(NCT):
                ps = pmm.tile([NP, L], F32, name=f"z_ps_{t}_{m}", tag="mm")
                for k in range(NCT):
                    lhs = wiz_sb[:, k * D + m * NP: k * D + (m + 1) * NP]
                    nc.tensor.matmul(ps[:], lhs, xn_t[t][k][:],
                                     start=(k == 0), stop=(k == NCT - 1))
                zt = zp.tile([NP, L + 3], BF16, name=f"z_{t}_{m}", tag="z")
                nc.gpsimd.tensor_copy(zt[:, 0:3], zhalo_prev[:, 3 * m:3 * m + 3])
                if m % 2 == 0:
                    nc.vector.tensor_copy(zt[:, 3:3 + L], ps[:])
                else:
                    nc.scalar.copy(zt[:, 3:3 + L], ps[:])
                nc.gpsimd.tensor_copy(zhalo_cur[:, 3 * m:3 * m + 3],
                                      zt[:, L:L + 3])
                # depthwise causal conv: tap0 via tensor_scalar (w + bias),
                # taps 1-3 via STT; all bf16
                zc0 = zc0p.tile([NP, L], BF16, name=f"zc_{t}_{m}", tag="zc")
                nc.vector.tensor_scalar(zc0[:], zt[:, 0:L],
                                        cw_sb[:, 0 * NCT + m:0 * NCT + m + 1],
                                        cb_sb[:, m:m + 1],
                                        op0=OP.mult, op1=OP.add)
                for k in (1, 2):
                    nc.vector.scalar_tensor_tensor(
                        zc0[:], zt[:, k:k + L],
                        cw_sb[:, k * NCT + m:k * NCT + m + 1],
                        zc0[:], op0=OP.mult, op1=OP.add)
                zcbm = zcbp.tile([NP, L], BF16, name=f"zcb_{t}_{m}", tag="zcb")
                nc.vector.scalar_tensor_tensor(
                    zcbm[:], zt[:, 3:3 + L],
                    cw_sb[:, 3 * NCT + m:3 * NCT + m + 1],
                    zc0[:], op0=OP.mult, op1=OP.add)
                zcb.append(zcbm)
            zhalo_prev = zhalo_cur
            zcb_t[t] = zcb

        def stage_C(t):
            """gates proj + tanh gates + alpha/beta + scans + spill, tile t.

            sigma(v) = (1+tanh(v/2))/2:
              tf = tanh(0.5*psf + gbf/2)      alpha = exp(msph*tf + msph)
              a2 = exp(msp*tf + msp)          beta' = exp(0.5*lu2 + ln 0.5)
              ti = tanh(0.5*psi + gbi/2)      xs = ((1+ti)*zcb) * beta'
            Scalar blocks are batched per table set (tanh/exp share one)."""
            zcb = zcb_t[t]
            tf = [None] * NCT
            ti = [None] * NCT
            for i in range(NCT):
                psf = pmm.tile([NP, L], F32, name=f"f_ps_{t}_{i}", tag="mm")
                for k in range(NCT):
                    lhs = wg_sb[:, k * H2 + i * NP: k * H2 + (i + 1) * NP]
                    nc.tensor.matmul(psf[:], lhs, zcb[k][:],
                                     start=(k == 0), stop=(k == NCT - 1))
                tfi = sfp.tile([NP, L], BF16, name=f"tf_{t}_{i}", tag="tf")
                nc.scalar.activation(tfi[:], psf[:], AF.Tanh, scale=0.5,
                                     bias=gbfh_sb[:, i:i + 1])
                tf[i] = tfi
                psi = pmm.tile([NP, L], F32, name=f"i_ps_{t}_{i}", tag="mm")
                for k in range(NCT):
                    lhs = wg_sb[:, k * H2 + D + i * NP: k * H2 + D + (i + 1) * NP]
                    nc.tensor.matmul(psi[:], lhs, zcb[k][:],
                                     start=(k == 0), stop=(k == NCT - 1))
                tii = sfp.tile([NP, L], BF16, name=f"ti_{t}_{i}", tag="ti")
                nc.scalar.activation(tii[:], psi[:], AF.Tanh, scale=0.5,
                                     bias=gbih_sb[:, i:i + 1])
                ti[i] = tii

            alpha = [None] * NCT
            a2 = [None] * NCT
            for i in range(NCT):
                al = ap_.tile([NP, L], F32, name=f"al_{t}_{i}", tag="alpha")
                nc.scalar.activation(al[:], tf[i][:], AF.Exp,
                                     scale=msph_sb[:, i:i + 1],
                                     bias=msph_sb[:, i:i + 1])
                alpha[i] = al
                a2i = bp_.tile([NP, L], F32, name=f"a2_{t}_{i}", tag="a2")
                nc.scalar.activation(a2i[:], tf[i][:], AF.Exp,
                                     scale=msp_sb[:, i:i + 1],
                                     bias=msp_sb[:, i:i + 1])
                a2[i] = a2i
            beta = [None] * NCT
            for i in range(NCT):
                lu2 = bp_.tile([NP, L], F32, name=f"lu2_{t}_{i}", tag="lu2")
                nc.scalar.activation(lu2[:], a2[i][:], AF.Ln, scale=-1.0,
                                     bias=onepb[:, 0:1])
                be = bp16.tile([NP, L], BF16, name=f"be_{t}_{i}", tag="beta")
                nc.scalar.activation(be[:], lu2[:], AF.Exp, scale=0.5,
                                     bias=lhalfb[:, 0:1])
                beta[i] = be

            h_w = hp.tile([NP, W], BF16, name=f"h_{t}", tag="h")
            ac_w = hp.tile([NP, W], BF16, name=f"ac_{t}", tag="ac")
            for i in range(NCT):
                sz = bp16.tile([NP, L], BF16, name=f"sz_{t}_{i}", tag="sz")
                nc.vector.scalar_tensor_tensor(sz[:], ti[i][:], 1.0,
                                               zcb[i][:],
                                               op0=OP.add, op1=OP.mult)
                xs = bp16.tile([NP, L], BF16, name=f"xs_{t}_{i}", tag="xs")
                nc.gpsimd.tensor_tensor(xs[:], sz[:], beta[i][:], OP.mult)

                hsl = h_w[:, i * L:(i + 1) * L]
                h_init = 0.0 if t == 0 else hlast[:, i:i + 1]
                nc.vector.tensor_tensor_scan(hsl, alpha[i][:], xs[:], h_init,
                                             op0=OP.mult, op1=OP.add)
                nc.gpsimd.tensor_copy(hlast[:, i:i + 1],
                                      h_w[:, (i + 1) * L - 1:(i + 1) * L])
                asl = ac_w[:, i * L:(i + 1) * L]
                a_init = 1.0 if t == 0 else alast[:, i:i + 1]
                nc.vector.tensor_tensor_scan(asl, alpha[i][:], ones_f[:, 0:L],
                                             a_init, op0=OP.mult, op1=OP.mult)
                nc.gpsimd.tensor_copy(alast[:, i:i + 1],
                                      ac_w[:, (i + 1) * L - 1:(i + 1) * L])
            nc.sync.dma_start(wide_in(h_d, t * L, L), h_w[:])
            nc.gpsimd.dma_start(wide_in(ac_d, t * L, L), ac_w[:])
            if t == n_tiles - 1:
                nc.scalar.dma_start(
                    carry_loc.ap()[0:1, :].rearrange("a (c p) -> p (a c)",
                                                     p=NP),
                    hlast[:])

        # -------- pipelined emission: A/B two tiles ahead of C --------
        x_t[0] = load_x(0)
        if n_tiles > 1:
            x_t[1] = load_x(1)
        stage_AB(0)
        if n_tiles > 1:
            stage_AB(1)
        for t in range(n_tiles):
            if t + 2 < n_tiles:
                x_t[t + 2] = load_x(t + 2)
                stage_AB(t + 2)
            stage_C(t)
            x_t[t] = None  # allow pool reuse

        p1.close()

        # ================= TRANSITION: gate projs + carry =================
        tr = ExitStack()
        wpool_g = tr.enter_context(tc.tile_pool(name="wg2", bufs=1))
        wig_sb = wpool_g.tile([NP, NCT * D], BF16, name="wig_sb")
        nc.sync.dma_start(wig_sb[:], wide_in(wig_d, 0, D))
        pmg = tr.enter_context(tc.tile_pool(name="pmg", bufs=4, space="PSUM"))
        gpool = tr.enter_context(tc.tile_pool(name="gpool", bufs=2))

        # pairwise carry exchange first: gate projs fill the PE while the
        # collective runs; carry reads ride the vector queue so the sync
        # queue's phase-2a reload DMAs are not head-of-line blocked.
        nc.gpsimd.collective_compute(
            "AllGather", OP.bypass,
            replica_groups=[[0, 1], [2, 3], [4, 5], [6, 7]],
            ins=[carry_loc.ap()], outs=[carry_gth.ap()])
        cg = cpool.tile([NP, NCT], F32, name="cg")
        nc.scalar.dma_start(
            cg[:],
            carry_gth.ap()[0:1, :].rearrange("a (c p) -> p (a c)", p=NP))
        carrym = cpool.tile([NP, NCT], F32, name="carrym")
        nc.vector.tensor_scalar(carrym[:], cg[:], cmask_sb[:, 0:1], None,
                                op0=OP.mult)

        for t in range(n_tiles):
            g_w = gpool.tile([NP, W], BF16, name=f"g_{t}", tag="g")
            for m in range(NCT):
                ps = pmg.tile([NP, L], F32, name=f"g_ps_{t}_{m}", tag="mmg")
                for k in range(NCT):
                    lhs = wig_sb[:, k * D + m * NP: k * D + (m + 1) * NP]
                    nc.tensor.matmul(ps[:], lhs, xn_t[t][k][:],
                                     start=(k == 0), stop=(k == NCT - 1))
                emit_gelu(g_w[:, m * L:(m + 1) * L], ps[:], gpool, f"g_{t}_{m}")
            nc.sync.dma_start(wide_in(g_d, t * L, L), g_w[:])
        tr.close()
        xn_scope.close()

        # =========================== PHASE 2a ===========================
        # long-lived pools first (outlive p2a; LIFO discipline)
        wpool_gr = top.enter_context(tc.tile_pool(name="wgr2", bufs=1))
        wgr_sb = wpool_gr.tile([NP, NCT * HID], BF16, name="wgr_sb")
        x1np = top.enter_context(tc.tile_pool(name="x1np",
                                              bufs=NCT * n_tiles))
        p2a = ExitStack()
        wpool_o = p2a.enter_context(tc.tile_pool(name="wo2", bufs=1))
        wo_sb = wpool_o.tile([NP, NCT * D], BF16, name="wo_sb")
        nc.sync.dma_start(wo_sb[:], wide_in(wo_d, 0, D))
        # grow weights start loading now (consumed in 2b)
        nc.scalar.dma_start(wgr_sb[:], wide_in(wgr_d, 0, HID))

        hp2 = p2a.enter_context(tc.tile_pool(name="hp2", bufs=1))
        grp = p2a.enter_context(tc.tile_pool(name="grp", bufs=1))
        xp2 = p2a.enter_context(tc.tile_pool(name="xp2", bufs=1))
        sp2b = p2a.enter_context(tc.tile_pool(name="sp2b", bufs=4))   # bf16
        sp2f = p2a.enter_context(tc.tile_pool(name="sp2f", bufs=1))   # f32
        ghp = p2a.enter_context(tc.tile_pool(name="ghp", bufs=NCT + 2))
        x1bp = p2a.enter_context(tc.tile_pool(name="x1bp", bufs=2))
        pmm2 = p2a.enter_context(tc.tile_pool(name="pmm2", bufs=4, space="PSUM"))
        pssq2 = p2a.enter_context(tc.tile_pool(name="pssq2", bufs=2,
                                               space="PSUM"))
        x1n_t = [[None] * NCT for _ in range(n_tiles)]

        for t in range(n_tiles):
            hr_w = hp2.tile([NP, W], BF16, name=f"hr_{t}", tag="hr")
            nc.sync.dma_start(hr_w[:], wide_in(h_d, t * L, L))
            ac_w = hp2.tile([NP, W], BF16, name=f"acr_{t}", tag="acr")
            nc.sync.dma_start(ac_w[:], wide_in(ac_d, t * L, L))
            gr_w = grp.tile([NP, W], BF16, name=f"grl_{t}", tag="grl")
            nc.sync.dma_start(gr_w[:], wide_in(g_d, t * L, L))
            x2_w = xp2.tile([NP, W], BF16, name=f"x2_{t}", tag="x2")
            nc.sync.dma_start(x2_w[:], wide_in(x_d, 3 + t * L, L))

            gh = []
            for i in range(NCT):
                hf = sp2b.tile([NP, L], BF16, name=f"hf_{t}_{i}", tag="hf")
                nc.vector.scalar_tensor_tensor(
                    hf[:], ac_w[:, i * L:(i + 1) * L], carrym[:, i:i + 1],
                    hr_w[:, i * L:(i + 1) * L], op0=OP.mult, op1=OP.add)
                ghi = ghp.tile([NP, L], BF16, name=f"gh_{t}_{i}", tag="gh")
                nc.vector.tensor_tensor(ghi[:], gr_w[:, i * L:(i + 1) * L],
                                        hf[:], OP.mult)
                gh.append(ghi)

            # output proj + residual -> x1 (bf16, wide) -> spill to DRAM
            x1_w = x1bp.tile([NP, W], BF16, name=f"x1_{t}", tag="x1")
            for m in range(NCT):
                ps = pmm2.tile([NP, L], F32, name=f"o_ps_{t}_{m}", tag="mm2")
                for k in range(NCT):
                    lhs = wo_sb[:, k * D + m * NP: k * D + (m + 1) * NP]
                    nc.tensor.matmul(ps[:], lhs, gh[k][:],
                                     start=(k == 0), stop=(k == NCT - 1))
                nc.vector.tensor_tensor(x1_w[:, m * L:(m + 1) * L], ps[:],
                                        x2_w[:, m * L:(m + 1) * L], OP.add)
            nc.sync.dma_start(wide_in(x1_d, t * L, L), x1_w[:])

            # rmsnorm2 -> x1n (bf16); squares on scalar (in-set everywhere)
            ssq = pssq2.tile([NP, L], F32, name=f"ssq2_{t}", tag="ssq2")
            for i in range(NCT):
                xsq = sp2b.tile([NP, L], BF16, name=f"x1sq_{t}_{i}", tag="x1sq")
                nc.scalar.activation(xsq[:], x1_w[:, i * L:(i + 1) * L],
                                     AF.Square)
                nc.tensor.matmul(ssq[:], ones_bf[:], xsq[:],
                                 start=(i == 0), stop=(i == NCT - 1))
            lssq = sp2f.tile([NP, L], F32, name=f"lssq2_{t}", tag="lssq2")
            nc.scalar.activation(lssq[:], ssq[:], AF.Ln, bias=epsb[:, 0:1])
            s2 = sp2b.tile([NP, L], BF16, name=f"s2_{t}", tag="s2")
            nc.scalar.activation(s2[:], lssq[:], AF.Exp, scale=-0.5)
            for i in range(NCT):
                xni = x1np.tile([NP, L], BF16, name=f"x1n_{t}_{i}", tag="x1n")
                nc.vector.tensor_tensor(xni[:], x1_w[:, i * L:(i + 1) * L],
                                        s2[:], OP.mult)
                x1n_t[t][i] = xni

        p2a.close()

        # =========================== PHASE 2b ===========================
        p2b = ExitStack()
        wpool_s = p2b.enter_context(tc.tile_pool(name="ws2", bufs=1))
        wsh_sb = wpool_s.tile([NP, 2 * NCT * D], BF16, name="wsh_sb")
        nc.sync.dma_start(wsh_sb[:], wide_in(wsh_d, 0, D))

        gvp = p2b.enter_context(tc.tile_pool(name="gvp", bufs=2 * NGT + 2))
        t2p = p2b.enter_context(tc.tile_pool(name="t2p", bufs=4))
        x1rp = p2b.enter_context(tc.tile_pool(name="x1rp", bufs=2))
        op_ = p2b.enter_context(tc.tile_pool(name="op", bufs=6))
        pmmg = p2b.enter_context(tc.tile_pool(name="pmmg", bufs=5, space="PSUM"))
        pmms = p2b.enter_context(tc.tile_pool(name="pmms", bufs=3, space="PSUM"))

        gv_t = [[None] * NGT for _ in range(n_tiles)]
        x1r_t = [None] * n_tiles

        def stage_grow(t):
            # prefetch x1 reload for the shrink residual
            x1r = x1rp.tile([NP, W], BF16, name=f"x1r_{t}", tag="x1r")
            nc.gpsimd.dma_start(x1r[:], wide_in(x1_d, t * L, L))
            x1r_t[t] = x1r
            for hm in range(NGT):
                psg = pmmg.tile([NP, L], F32, name=f"g2_ps_{t}_{hm}", tag="mmg2")
                for k in range(NCT):
                    lhs = wgr_sb[:, k * HID + hm * NP: k * HID + (hm + 1) * NP]
                    nc.tensor.matmul(psg[:], lhs, x1n_t[t][k][:],
                                     start=(k == 0), stop=(k == NCT - 1))
                psv = pmmg.tile([NP, L], F32, name=f"v_ps_{t}_{hm}", tag="mmg2")
                for k in range(NCT):
                    lhs = wgr_sb[:, k * HID + H2 + hm * NP:
                                 k * HID + H2 + (hm + 1) * NP]
                    nc.tensor.matmul(psv[:], lhs, x1n_t[t][k][:],
                                     start=(k == 0), stop=(k == NCT - 1))
                t2 = t2p.tile([NP, L], BF16, name=f"t2_{t}_{hm}", tag="t2")
                emit_gelu(t2[:], psg[:], t2p, f"t2_{t}_{hm}")
                gvi = gvp.tile([NP, L], BF16, name=f"gv_{t}_{hm}", tag="gv")
                nc.vector.tensor_tensor(gvi[:], t2[:], psv[:], OP.mult)
                gv_t[t][hm] = gvi

        def stage_shrink(t):
            for m in range(NCT):
                ps = pmms.tile([NP, L], F32, name=f"s_ps_{t}_{m}", tag="mms")
                for k in range(2 * NCT):
                    lhs = wsh_sb[:, k * D + m * NP: k * D + (m + 1) * NP]
                    nc.tensor.matmul(ps[:], lhs, gv_t[t][k][:],
                                     start=(k == 0), stop=(k == 2 * NCT - 1))
                om = op_.tile([NP, L], F32, name=f"out_{t}_{m}", tag="out")
                nc.vector.tensor_tensor(om[:], ps[:],
                                        x1r_t[t][:, m * L:(m + 1) * L], OP.add)
                nc.sync.dma_start(
                    out_d.ap()[m * NP:(m + 1) * NP, t * L:(t + 1) * L], om[:])

        stage_grow(0)
        for t in range(1, n_tiles):
            stage_grow(t)
            stage_shrink(t - 1)
        stage_shrink(n_tiles - 1)
        p2b.close()

    nc.compile()
    return nc


def host_prepare(inputs, T_core, n_cores=N_CORES):
    """Build per-core in_maps from full inputs."""
    x = np.asarray(inputs["x"], np.float32)            # [B, T, D]
    B, T, _ = x.shape
    halves = n_cores // B
    assert T == halves * T_core

    gam1 = np.asarray(inputs["hawk_norm_gamma"], np.float32)
    gam2 = np.asarray(inputs["gmlp_norm_gamma"], np.float32)
    scale1 = gam1 * np.sqrt(D)
    scale2 = gam2 * np.sqrt(D)

    wi = (np.asarray(inputs["input_w"], np.float32) * scale1[None, :]).T
    wg = np.asarray(inputs["gates_w"], np.float32).T
    wo = np.asarray(inputs["output_w"], np.float32).T
    wgr = (np.asarray(inputs["grow_w"], np.float32) * scale2[None, :]).T
    wsh = np.asarray(inputs["shrink_w"], np.float32).T

    fb = np.asarray(inputs["forget_base"], np.float64)
    msp = (-8.0 * np.log1p(np.exp(fb))).astype(np.float32)

    def chan_layout(v):  # [D] -> [128, 8] with [p, i] = v[128*i + p]
        return np.ascontiguousarray(v.reshape(NCT, NP).T)

    gb = np.asarray(inputs["gates_b"], np.float32)
    cw = np.asarray(inputs["conv_w"], np.float32)[:, 0, :]   # [D, K]
    cb = np.asarray(inputs["conv_b"], np.float32)

    shared = {
        "wig": np.ascontiguousarray(wi[:, :D]).astype(_BF),
        "wiz": np.ascontiguousarray(wi[:, D:]).astype(_BF),
        "wg": wg.astype(_BF), "wo": wo.astype(_BF),
        "wgr": wgr.astype(_BF), "wsh": wsh.astype(_BF),
        "msp": chan_layout(msp), "msph": chan_layout(0.5 * msp),
        "gbfh": chan_layout(0.5 * gb[:D]), "gbih": chan_layout(0.5 * gb[D:]),
        "cw": np.concatenate([chan_layout(cw[:, k]) for k in range(KCONV)],
                             axis=1),
        "cb": chan_layout(cb),
    }
    in_maps = []
    for core in range(n_cores):
        b, h = core // halves, core % halves
        xf = np.zeros((D, 3 + T_core), np.float32)
        xf[:, 3:] = x[b, h * T_core:(h + 1) * T_core, :].T
        if h > 0:
            xf[:, 0:3] = x[b, h * T_core - 3:h * T_core, :].T
        m = dict(shared)
        m["x"] = xf.astype(_BF)
        m["cmask"] = np.full((NP, 1), 1.0 if h > 0 else 0.0, np.float32)
        in_maps.append(m)
    return in_maps


def assemble_output(results, B, T, T_core, n_cores=N_CORES):
    halves = n_cores // B
    out = np.empty((B, T, D), np.float32)
    for core in range(n_cores):
        b, h = core // halves, core % halves
        out[b, h * T_core:(h + 1) * T_core, :] = results[core]["out"].T
    return out


_PROG_CACHE = {}


def kernel(**inputs) -> np.ndarray:
    x = np.asarray(inputs["x"])
    B, T, _ = x.shape
    T_core = T * B // N_CORES
    L = 512 if T_core % 512 == 0 else T_core // 4
    key = (T_core, L)
    if key not in _PROG_CACHE:
        _PROG_CACHE[key] = build_program(T_core, L)
    nc = _PROG_CACHE[key]
    in_maps = host_prepare(inputs, T_core)
    res = run_bass_kernel_spmd(nc, in_maps, list(range(N_CORES)))
    return assemble_output(res.results, B, T, T_core)


# revision 36
# speedup vs baseline: 1.0275x; 1.0232x over previous
"""Griffin block (Hawk RG-LRU + GatedMLP) Trainium2 Bass kernel, v4.

Sharding: 8 chunks = 4 batches x 2 time-halves, one per NeuronCore.
Per-core layout is feature-major ([channels, tokens]); everything bf16
except the recurrence coefficients (alpha stays f32) and the final
residual adds (psum f32 + bf16 carrier).

v4 notes:
  - sigmoids replaced with tanh (sigma(x) = (1+tanh(x/2))/2 folded into
    downstream scales/biases) so the forget/input gates, alpha and a2
    exps all live in ONE activation-table set (exp_and_others has both
    exp and tanh); Ln resolves to natural_log_exp_and_others via the
    table patch in this file, so the beta chain stays resident too.
  - per-(tensor, tile) wide SBUF tiles with a single batched DMA each
    (a dma_start costs ~0.6us of engine time; v3 issued ~450 of them).
  - phase 1 runs a two-tile software pipeline; hawk gate projection,
    gelu and the carry AllGather fill the scan tail (transition);
    phase 2a (output proj + residual + norm2) then 2b (grow/shrink).
"""

import numpy as np
import ml_dtypes
from contextlib import ExitStack

import concourse.bass as bass
import concourse.bacc as bacc
import concourse.tile as tile
from concourse import mybir
from concourse.bass_utils import run_bass_kernel_spmd

# The act-table-load pass maps each activation to the FIRST table set that
# contains it: Ln -> "natural_log" (ln only), Exp -> "exp_and_others", which
# thrashes a table load on every Ln<->Exp alternation.  Dropping `ln` from
# the ln-only set makes Ln resolve to "natural_log_exp_and_others" (has BOTH
# ln and exp), so ln/exp stretches share one resident set.  Set ids keep
# their act_info.json positions; the hardware tables are unchanged.
import concourse.hw_specs as _hw_specs


def _patched_act_tables(arch):
    tabs = _hw_specs.get_activation_tables(arch)
    out = {}
    for name, fns in tabs.items():
        if name == "natural_log":
            fns = fns - {mybir.ActivationFunctionType.Ln}
        out[name] = fns
    return out


bacc.get_activation_tables = _patched_act_tables
import concourse.bass_interp as _bass_interp
_bass_interp.get_activation_tables = _patched_act_tables

F32 = mybir.dt.float32
BF16 = mybir.dt.bfloat16
AF = mybir.ActivationFunctionType
OP = mybir.AluOpType

D = 1024
NP = 128          # partitions
NCT = D // NP     # channel tiles = 8
KCONV = 4
N_CORES = 8

_BF = ml_dtypes.bfloat16


def build_program(T_core: int, L: int, gelu_approx: bool = False):
    """Emit the SPMD program. T_core tokens per core, token tile L."""
    assert T_core % L == 0
    n_tiles = T_core // L
    H2 = 2 * D        # hawk proj width (2048)
    HID = 2 * H2      # gmlp hidden rows (4096): gate2 [0:2048), v [2048:4096)
    NGT = H2 // NP    # 16 gate/v ctiles
    W = NCT * L       # wide tile free size

    nc = bacc.Bacc("TRN2", target_bir_lowering=False, debug=False,
                   num_devices=N_CORES)

    # ---- DRAM parameters (per-core data via in_maps) ----
    x_d = nc.dram_tensor("x", [D, 3 + T_core], BF16, kind="ExternalInput")
    wiz_d = nc.dram_tensor("wiz", [D, D], BF16, kind="ExternalInput")    # z rows of input_w.T (gamma folded)
    wig_d = nc.dram_tensor("wig", [D, D], BF16, kind="ExternalInput")    # gate rows
    wg_d = nc.dram_tensor("wg", [D, H2], BF16, kind="ExternalInput")     # gates_w.T
    wo_d = nc.dram_tensor("wo", [D, D], BF16, kind="ExternalInput")      # output_w.T
    wgr_d = nc.dram_tensor("wgr", [D, HID], BF16, kind="ExternalInput")  # grow_w.T (gamma folded)
    wsh_d = nc.dram_tensor("wsh", [H2, D], BF16, kind="ExternalInput")   # shrink_w.T
    # per-channel params, laid out [partition, ch_tile]
    msp_d = nc.dram_tensor("msp", [NP, NCT], F32, kind="ExternalInput")    # -8*softplus(fb)
    msph_d = nc.dram_tensor("msph", [NP, NCT], F32, kind="ExternalInput")  # msp/2
    gbfh_d = nc.dram_tensor("gbfh", [NP, NCT], F32, kind="ExternalInput")  # gates_b[:D]/2
    gbih_d = nc.dram_tensor("gbih", [NP, NCT], F32, kind="ExternalInput")  # gates_b[D:]/2
    cw_d = nc.dram_tensor("cw", [NP, KCONV * NCT], F32, kind="ExternalInput")  # conv w taps
    cb_d = nc.dram_tensor("cb", [NP, NCT], F32, kind="ExternalInput")      # conv bias
    cmask_d = nc.dram_tensor("cmask", [NP, 1], F32, kind="ExternalInput")  # 1.0 iff second half

    out_d = nc.dram_tensor("out", [D, T_core], F32, kind="ExternalOutput")

    # ---- internal DRAM scratch ----
    h_d = nc.dram_tensor("h_spill", [D, T_core], BF16)
    ac_d = nc.dram_tensor("ac_spill", [D, T_core], BF16)
    x1_d = nc.dram_tensor("x1_spill", [D, T_core], BF16)
    g_d = nc.dram_tensor("g_spill", [D, T_core], BF16)
    carry_loc = nc.dram_tensor("carry_loc", [1, D], F32)
    carry_gth = nc.dram_tensor("carry_gth", [2, D], F32)

    def wide_in(dram, c0, w):
        """[D', c0:c0+w] -> [128, (D'/128)*w] AP (ctile-major free dim)."""
        return dram.ap()[:, c0:c0 + w].rearrange("(c p) t -> p c t", p=NP)

    with tile.TileContext(nc) as tc, ExitStack() as top:
        # ------- persistent small constants -------
        cpool = top.enter_context(tc.tile_pool(name="consts", bufs=1))
        ones_bf = cpool.tile([NP, NP], BF16, name="ones_bf")
        nc.vector.memset(ones_bf[:], 1.0)
        ones_f = cpool.tile([NP, L], F32, name="ones_f")
        nc.vector.memset(ones_f[:], 1.0)
        msp_sb = cpool.tile([NP, NCT], F32, name="msp_sb")
        nc.sync.dma_start(msp_sb[:], msp_d.ap()[:, :])
        msph_sb = cpool.tile([NP, NCT], F32, name="msph_sb")
        nc.sync.dma_start(msph_sb[:], msph_d.ap()[:, :])
        gbfh_sb = cpool.tile([NP, NCT], F32, name="gbfh_sb")
        nc.sync.dma_start(gbfh_sb[:], gbfh_d.ap()[:, :])
        gbih_sb = cpool.tile([NP, NCT], F32, name="gbih_sb")
        nc.sync.dma_start(gbih_sb[:], gbih_d.ap()[:, :])
        cw_sb = cpool.tile([NP, KCONV * NCT], F32, name="cw_sb")
        nc.sync.dma_start(cw_sb[:], cw_d.ap()[:, :])
        cb_sb = cpool.tile([NP, NCT], F32, name="cb_sb")
        nc.sync.dma_start(cb_sb[:], cb_d.ap()[:, :])
        cmask_sb = cpool.tile([NP, 1], F32, name="cmask_sb")
        nc.sync.dma_start(cmask_sb[:], cmask_d.ap()[:, :])
        hlast = cpool.tile([NP, NCT], F32, name="hlast")
        alast = cpool.tile([NP, NCT], F32, name="alast")
        epsb = cpool.tile([NP, 1], F32, name="epsb")
        nc.vector.memset(epsb[:], 1e-20)
        onepb = cpool.tile([NP, 1], F32, name="onepb")
        nc.vector.memset(onepb[:], 1.0 + 1e-6)
        lhalfb = cpool.tile([NP, 1], F32, name="lhalfb")
        nc.vector.memset(lhalfb[:], float(np.log(0.5)))

        def emit_gelu(out_ap, ps, pool, tag):
            """gelu(ps) -> out; sim lacks Gelu so approx mode builds it."""
            if gelu_approx:
                sg = pool.tile([NP, ps.shape[-1]], F32, name=f"sg_{tag}",
                               tag="gelu_sg")
                nc.scalar.activation(sg[:], ps, AF.Sigmoid, scale=1.702)
                nc.vector.tensor_tensor(out_ap, ps, sg[:], OP.mult)
            else:
                nc.scalar.activation(out_ap, ps, AF.Gelu)

        # xn persists through the transition (gate projs consume it)
        xn_scope = ExitStack()
        xnp = xn_scope.enter_context(
            tc.tile_pool(name="xnp", bufs=NCT * n_tiles))
        xn_t = [None] * n_tiles          # [t][i] -> [NP, L] bf16

        # =========================== PHASE 1 ===========================
        p1 = ExitStack()
        wpool = p1.enter_context(tc.tile_pool(name="w1", bufs=1))
        wiz_sb = wpool.tile([NP, NCT * D], BF16, name="wiz_sb")
        wg_sb = wpool.tile([NP, NCT * H2], BF16, name="wg_sb")

        xp = p1.enter_context(tc.tile_pool(name="xp", bufs=3))
        hxp = p1.enter_context(tc.tile_pool(name="hxp", bufs=NCT + 1))
        sbp = p1.enter_context(tc.tile_pool(name="sbp", bufs=3))   # bf16 temps
        sfp32 = p1.enter_context(tc.tile_pool(name="sfp32", bufs=2))  # f32 temps
        zp = p1.enter_context(tc.tile_pool(name="zp", bufs=NCT + 2))
        zc0p = p1.enter_context(tc.tile_pool(name="zc0p", bufs=4))
        zcbp = p1.enter_context(tc.tile_pool(name="zcbp", bufs=3 * NCT))
        sfp = p1.enter_context(tc.tile_pool(name="sfp", bufs=4))
        ap_ = p1.enter_context(tc.tile_pool(name="ap", bufs=3))
        bp_ = p1.enter_context(tc.tile_pool(name="bp", bufs=2))    # f32 a2/lu2
        bp16 = p1.enter_context(tc.tile_pool(name="bp16", bufs=3))  # bf16 b/sz/xs
        hp = p1.enter_context(tc.tile_pool(name="hp", bufs=1))      # wide h/ac
        zhp = p1.enter_context(tc.tile_pool(name="zhp", bufs=2))
        pmm = p1.enter_context(tc.tile_pool(name="pmm", bufs=5, space="PSUM"))
        pssq = p1.enter_context(tc.tile_pool(name="pssq", bufs=2, space="PSUM"))

        def load_x(t):
            xt = xp.tile([NP, W], BF16, name=f"x_{t}", tag="x")
            nc.sync.dma_start(xt[:], wide_in(x_d, 3 + t * L, L))
            return xt

        def norm_tiles(x_w, w, tag, pool, xsl):
            """s = exp(-0.5*ln(ssq)) = 1/||x||; xn = x*s (bf16).
            x_w wide tile; xsl(i) -> slice of ctile i."""
            ssq = pssq.tile([NP, w], F32, name=f"ssq_{tag}", tag="ssq")
            for i in range(NCT):
                xsq = sbp.tile([NP, w], BF16, name=f"xsq_{tag}_{i}", tag="xsq")
                nc.scalar.activation(xsq[:], xsl(x_w, i), AF.Square)
                nc.tensor.matmul(ssq[:], ones_bf[:], xsq[:],
                                 start=(i == 0), stop=(i == NCT - 1))
            lssq = sfp32.tile([NP, w], F32, name=f"lssq_{tag}", tag="lssq")
            nc.scalar.activation(lssq[:], ssq[:], AF.Ln, bias=epsb[:, 0:1])
            s = sbp.tile([NP, w], BF16, name=f"s_{tag}", tag="s")
            nc.scalar.activation(s[:], lssq[:], AF.Exp, scale=-0.5)
            xn = []
            for i in range(NCT):
                t_ = pool.tile([NP, w], BF16, name=f"xn_{tag}_{i}", tag="xn")
                nc.vector.tensor_tensor(t_[:], xsl(x_w, i), s[:], OP.mult)
                xn.append(t_)
            return xn

        def xsl_L(x_w, i):
            return x_w[:, i * L:(i + 1) * L]

        # ---- halo z: conv inputs for the 3 tokens before this chunk ----
        xh = xp.tile([NP, 3 * NCT], BF16, name="xh", tag="xh")
        nc.sync.dma_start(xh[:], wide_in(x_d, 0, 3))
        nc.sync.dma_start(wiz_sb[:], wide_in(wiz_d, 0, D))
        nc.scalar.dma_start(wg_sb[:], wide_in(wg_d, 0, H2))
        xnh = norm_tiles(xh, 3, "h", hxp, lambda w_, i: w_[:, 3 * i:3 * i + 3])
        zhalo_prev = zhp.tile([NP, 3 * NCT], BF16, name="zhalo_h", tag="zhalo")
        for m in range(NCT):
            ps = pmm.tile([NP, 3], F32, name=f"zh_ps_{m}", tag="mm")
            for k in range(NCT):
                lhs = wiz_sb[:, k * D + m * NP: k * D + (m + 1) * NP]
                nc.tensor.matmul(ps[:], lhs, xnh[k][:],
                                 start=(k == 0), stop=(k == NCT - 1))
            nc.vector.tensor_copy(zhalo_prev[:, 3 * m:3 * m + 3], ps[:])

        x_t = [None] * n_tiles
        zcb_t = [None] * n_tiles

        def stage_AB(t):
            """norm + z-proj + conv for tile t."""
            nonlocal zhalo_prev
            xn_t[t] = norm_tiles(x_t[t], L, f"t{t}", xnp, xsl_L)
            zhalo_cur = zhp.tile([NP, 3 * NCT], BF16, name=f"zhalo_{t}",
                                 tag="zhalo")
            zcb = []
            for m in range(NCT):
                ps = pmm.tile([NP, L], F32, name=f"z_ps_{t}_{m}", tag="mm")
                for k in range(NCT):
                    lhs = wiz_sb[:, k * D + m * NP: k * D + (m + 1) * NP]
                    nc.tensor.matmul(ps[:], lhs, xn_t[t][k][:],
                                     start=(k == 0), stop=(k == NCT - 1))
                zt = zp.tile([NP, L + 3], BF16, name=f"z_{t}_{m}", tag="z")
                nc.gpsimd.tensor_copy(zt[:, 0:3], zhalo_prev[:, 3 * m:3 * m + 3])
                nc.vector.tensor_copy(zt[:, 3:3 + L], ps[:])
                nc.gpsimd.tensor_copy(zhalo_cur[:, 3 * m:3 * m + 3],
                                      zt[:, L:L + 3])
                # depthwise causal conv: tap0 via tensor_scalar (w + bias),
                # taps 1-3 via STT; all bf16
                zc0 = zc0p.tile([NP, L], BF16, name=f"zc_{t}_{m}", tag="zc")
                nc.vector.tensor_scalar(zc0[:], zt[:, 0:L],
                                        cw_sb[:, 0 * NCT + m:0 * NCT + m + 1],
                                        cb_sb[:, m:m + 1],
                                        op0=OP.mult, op1=OP.add)
                for k in (1, 2):
                    nc.vector.scalar_tensor_tensor(
                        zc0[:], zt[:, k:k + L],
                        cw_sb[:, k * NCT + m:k * NCT + m + 1],
                        zc0[:], op0=OP.mult, op1=OP.add)
                zcbm = zcbp.tile([NP, L], BF16, name=f"zcb_{t}_{m}", tag="zcb")
                nc.vector.scalar_tensor_tensor(
                    zcbm[:], zt[:, 3:3 + L],
                    cw_sb[:, 3 * NCT + m:3 * NCT + m + 1],
                    zc0[:], op0=OP.mult, op1=OP.add)
                zcb.append(zcbm)
            zhalo_prev = zhalo_cur
            zcb_t[t] = zcb

        def stage_C(t):
            """gates proj + tanh gates + alpha/beta + scans + spill, tile t.

            sigma(v) = (1+tanh(v/2))/2:
              tf = tanh(0.5*psf + gbf/2)      alpha = exp(msph*tf + msph)
              a2 = exp(msp*tf + msp)          beta' = exp(0.5*lu2 + ln 0.5)
              ti = tanh(0.5*psi + gbi/2)      xs = ((1+ti)*zcb) * beta'
            Scalar blocks are batched per table set (tanh/exp share one)."""
            zcb = zcb_t[t]
            tf = [None] * NCT
            ti = [None] * NCT
            for i in range(NCT):
                psf = pmm.tile([NP, L], F32, name=f"f_ps_{t}_{i}", tag="mm")
                for k in range(NCT):
                    lhs = wg_sb[:, k * H2 + i * NP: k * H2 + (i + 1) * NP]
                    nc.tensor.matmul(psf[:], lhs, zcb[k][:],
                                     start=(k == 0), stop=(k == NCT - 1))
                tfi = sfp.tile([NP, L], BF16, name=f"tf_{t}_{i}", tag="tf")
                nc.scalar.activation(tfi[:], psf[:], AF.Tanh, scale=0.5,
                                     bias=gbfh_sb[:, i:i + 1])
                tf[i] = tfi
                psi = pmm.tile([NP, L], F32, name=f"i_ps_{t}_{i}", tag="mm")
                for k in range(NCT):
                    lhs = wg_sb[:, k * H2 + D + i * NP: k * H2 + D + (i + 1) * NP]
                    nc.tensor.matmul(psi[:], lhs, zcb[k][:],
                                     start=(k == 0), stop=(k == NCT - 1))
                tii = sfp.tile([NP, L], BF16, name=f"ti_{t}_{i}", tag="ti")
                nc.scalar.activation(tii[:], psi[:], AF.Tanh, scale=0.5,
                                     bias=gbih_sb[:, i:i + 1])
                ti[i] = tii

            alpha = [None] * NCT
            a2 = [None] * NCT
            for i in range(NCT):
                al = ap_.tile([NP, L], F32, name=f"al_{t}_{i}", tag="alpha")
                nc.scalar.activation(al[:], tf[i][:], AF.Exp,
                                     scale=msph_sb[:, i:i + 1],
                                     bias=msph_sb[:, i:i + 1])
                alpha[i] = al
                a2i = bp_.tile([NP, L], F32, name=f"a2_{t}_{i}", tag="a2")
                nc.scalar.activation(a2i[:], tf[i][:], AF.Exp,
                                     scale=msp_sb[:, i:i + 1],
                                     bias=msp_sb[:, i:i + 1])
                a2[i] = a2i
            beta = [None] * NCT
            for i in range(NCT):
                lu2 = bp_.tile([NP, L], F32, name=f"lu2_{t}_{i}", tag="lu2")
                nc.scalar.activation(lu2[:], a2[i][:], AF.Ln, scale=-1.0,
                                     bias=onepb[:, 0:1])
                be = bp16.tile([NP, L], BF16, name=f"be_{t}_{i}", tag="beta")
                nc.scalar.activation(be[:], lu2[:], AF.Exp, scale=0.5,
                                     bias=lhalfb[:, 0:1])
                beta[i] = be

            h_w = hp.tile([NP, W], BF16, name=f"h_{t}", tag="h")
            ac_w = hp.tile([NP, W], BF16, name=f"ac_{t}", tag="ac")
            for i in range(NCT):
                sz = bp16.tile([NP, L], BF16, name=f"sz_{t}_{i}", tag="sz")
                nc.vector.scalar_tensor_tensor(sz[:], ti[i][:], 1.0,
                                               zcb[i][:],
                                               op0=OP.add, op1=OP.mult)
                xs = bp16.tile([NP, L], BF16, name=f"xs_{t}_{i}", tag="xs")
                nc.gpsimd.tensor_tensor(xs[:], sz[:], beta[i][:], OP.mult)

                hsl = h_w[:, i * L:(i + 1) * L]
                h_init = 0.0 if t == 0 else hlast[:, i:i + 1]
                nc.vector.tensor_tensor_scan(hsl, alpha[i][:], xs[:], h_init,
                                             op0=OP.mult, op1=OP.add)
                nc.gpsimd.tensor_copy(hlast[:, i:i + 1],
                                      h_w[:, (i + 1) * L - 1:(i + 1) * L])
                asl = ac_w[:, i * L:(i + 1) * L]
                a_init = 1.0 if t == 0 else alast[:, i:i + 1]
                nc.vector.tensor_tensor_scan(asl, alpha[i][:], ones_f[:, 0:L],
                                             a_init, op0=OP.mult, op1=OP.mult)
                nc.gpsimd.tensor_copy(alast[:, i:i + 1],
                                      ac_w[:, (i + 1) * L - 1:(i + 1) * L])
            nc.sync.dma_start(wide_in(h_d, t * L, L), h_w[:])
            nc.gpsimd.dma_start(wide_in(ac_d, t * L, L), ac_w[:])
            if t == n_tiles - 1:
                nc.scalar.dma_start(
                    carry_loc.ap()[0:1, :].rearrange("a (c p) -> p (a c)",
                                                     p=NP),
                    hlast[:])

        # -------- pipelined emission: A/B two tiles ahead of C --------
        x_t[0] = load_x(0)
        if n_tiles > 1:
            x_t[1] = load_x(1)
        stage_AB(0)
        if n_tiles > 1:
            stage_AB(1)
        for t in range(n_tiles):
            if t + 2 < n_tiles:
                x_t[t + 2] = load_x(t + 2)
                stage_AB(t + 2)
            stage_C(t)
            x_t[t] = None  # allow pool reuse

        p1.close()

        # ================= TRANSITION: gate projs + carry =================
        tr = ExitStack()
        wpool_g = tr.enter_context(tc.tile_pool(name="wg2", bufs=1))
        wig_sb = wpool_g.tile([NP, NCT * D], BF16, name="wig_sb")
        nc.sync.dma_start(wig_sb[:], wide_in(wig_d, 0, D))
        pmg = tr.enter_context(tc.tile_pool(name="pmg", bufs=4, space="PSUM"))
        gpool = tr.enter_context(tc.tile_pool(name="gpool", bufs=2))

        # pairwise carry exchange first: gate projs fill the PE while the
        # collective runs; carry reads ride the vector queue so the sync
        # queue's phase-2a reload DMAs are not head-of-line blocked.
        nc.gpsimd.collective_compute(
            "AllGather", OP.bypass,
            replica_groups=[[0, 1], [2, 3], [4, 5], [6, 7]],
            ins=[carry_loc.ap()], outs=[carry_gth.ap()])
        cg = cpool.tile([NP, NCT], F32, name="cg")
        nc.scalar.dma_start(
            cg[:],
            carry_gth.ap()[0:1, :].rearrange("a (c p) -> p (a c)", p=NP))
        carrym = cpool.tile([NP, NCT], F32, name="carrym")
        nc.vector.tensor_scalar(carrym[:], cg[:], cmask_sb[:, 0:1], None,
                                op0=OP.mult)

        for t in range(n_tiles):
            g_w = gpool.tile([NP, W], BF16, name=f"g_{t}", tag="g")
            for m in range(NCT):
                ps = pmg.tile([NP, L], F32, name=f"g_ps_{t}_{m}", tag="mmg")
                for k in range(NCT):
                    lhs = wig_sb[:, k * D + m * NP: k * D + (m + 1) * NP]
                    nc.tensor.matmul(ps[:], lhs, xn_t[t][k][:],
                                     start=(k == 0), stop=(k == NCT - 1))
                emit_gelu(g_w[:, m * L:(m + 1) * L], ps[:], gpool, f"g_{t}_{m}")
            nc.sync.dma_start(wide_in(g_d, t * L, L), g_w[:])
        tr.close()
        xn_scope.close()

        # =========================== PHASE 2a ===========================
        # long-lived pools first (outlive p2a; LIFO discipline)
        wpool_gr = top.enter_context(tc.tile_pool(name="wgr2", bufs=1))
        wgr_sb = wpool_gr.tile([NP, NCT * HID], BF16, name="wgr_sb")
        x1np = top.enter_context(tc.tile_pool(name="x1np",
                                              bufs=NCT * n_tiles))
        p2a = ExitStack()
        wpool_o = p2a.enter_context(tc.tile_pool(name="wo2", bufs=1))
        wo_sb = wpool_o.tile([NP, NCT * D], BF16, name="wo_sb")
        nc.sync.dma_start(wo_sb[:], wide_in(wo_d, 0, D))
        # grow weights start loading now (consumed in 2b)
        nc.scalar.dma_start(wgr_sb[:], wide_in(wgr_d, 0, HID))

        hp2 = p2a.enter_context(tc.tile_pool(name="hp2", bufs=1))
        grp = p2a.enter_context(tc.tile_pool(name="grp", bufs=1))
        xp2 = p2a.enter_context(tc.tile_pool(name="xp2", bufs=1))
        sp2b = p2a.enter_context(tc.tile_pool(name="sp2b", bufs=4))   # bf16
        sp2f = p2a.enter_context(tc.tile_pool(name="sp2f", bufs=1))   # f32
        ghp = p2a.enter_context(tc.tile_pool(name="ghp", bufs=NCT + 2))
        x1bp = p2a.enter_context(tc.tile_pool(name="x1bp", bufs=2))
        pmm2 = p2a.enter_context(tc.tile_pool(name="pmm2", bufs=4, space="PSUM"))
        pssq2 = p2a.enter_context(tc.tile_pool(name="pssq2", bufs=2,
                                               space="PSUM"))
        x1n_t = [[None] * NCT for _ in range(n_tiles)]

        for t in range(n_tiles):
            hr_w = hp2.tile([NP, W], BF16, name=f"hr_{t}", tag="hr")
            nc.sync.dma_start(hr_w[:], wide_in(h_d, t * L, L))
            ac_w = hp2.tile([NP, W], BF16, name=f"acr_{t}", tag="acr")
            nc.sync.dma_start(ac_w[:], wide_in(ac_d, t * L, L))
            gr_w = grp.tile([NP, W], BF16, name=f"grl_{t}", tag="grl")
            nc.sync.dma_start(gr_w[:], wide_in(g_d, t * L, L))
            x2_w = xp2.tile([NP, W], BF16, name=f"x2_{t}", tag="x2")
            nc.sync.dma_start(x2_w[:], wide_in(x_d, 3 + t * L, L))

            gh = []
            for i in range(NCT):
                hf = sp2b.tile([NP, L], BF16, name=f"hf_{t}_{i}", tag="hf")
                nc.vector.scalar_tensor_tensor(
                    hf[:], ac_w[:, i * L:(i + 1) * L], carrym[:, i:i + 1],
                    hr_w[:, i * L:(i + 1) * L], op0=OP.mult, op1=OP.add)
                ghi = ghp.tile([NP, L], BF16, name=f"gh_{t}_{i}", tag="gh")
                nc.vector.tensor_tensor(ghi[:], gr_w[:, i * L:(i + 1) * L],
                                        hf[:], OP.mult)
                gh.append(ghi)

            # output proj + residual -> x1 (bf16, wide) -> spill to DRAM
            x1_w = x1bp.tile([NP, W], BF16, name=f"x1_{t}", tag="x1")
            for m in range(NCT):
                ps = pmm2.tile([NP, L], F32, name=f"o_ps_{t}_{m}", tag="mm2")
                for k in range(NCT):
                    lhs = wo_sb[:, k * D + m * NP: k * D + (m + 1) * NP]
                    nc.tensor.matmul(ps[:], lhs, gh[k][:],
                                     start=(k == 0), stop=(k == NCT - 1))
                nc.vector.tensor_tensor(x1_w[:, m * L:(m + 1) * L], ps[:],
                                        x2_w[:, m * L:(m + 1) * L], OP.add)
            nc.sync.dma_start(wide_in(x1_d, t * L, L), x1_w[:])

            # rmsnorm2 -> x1n (bf16); squares on scalar (in-set everywhere)
            ssq = pssq2.tile([NP, L], F32, name=f"ssq2_{t}", tag="ssq2")
            for i in range(NCT):
                xsq = sp2b.tile([NP, L], BF16, name=f"x1sq_{t}_{i}", tag="x1sq")
                nc.scalar.activation(xsq[:], x1_w[:, i * L:(i + 1) * L],
                                     AF.Square)
                nc.tensor.matmul(ssq[:], ones_bf[:], xsq[:],
                                 start=(i == 0), stop=(i == NCT - 1))
            lssq = sp2f.tile([NP, L], F32, name=f"lssq2_{t}", tag="lssq2")
            nc.scalar.activation(lssq[:], ssq[:], AF.Ln, bias=epsb[:, 0:1])
            s2 = sp2b.tile([NP, L], BF16, name=f"s2_{t}", tag="s2")
            nc.scalar.activation(s2[:], lssq[:], AF.Exp, scale=-0.5)
            for i in range(NCT):
                xni = x1np.tile([NP, L], BF16, name=f"x1n_{t}_{i}", tag="x1n")
                nc.vector.tensor_tensor(xni[:], x1_w[:, i * L:(i + 1) * L],
                                        s2[:], OP.mult)
                x1n_t[t][i] = xni

        p2a.close()

        # =========================== PHASE 2b ===========================
        p2b = ExitStack()
        wpool_s = p2b.enter_context(tc.tile_pool(name="ws2", bufs=1))
        wsh_sb = wpool_s.tile([NP, 2 * NCT * D], BF16, name="wsh_sb")
        nc.sync.dma_start(wsh_sb[:], wide_in(wsh_d, 0, D))

        gvp = p2b.enter_context(tc.tile_pool(name="gvp", bufs=2 * NGT + 2))
        t2p = p2b.enter_context(tc.tile_pool(name="t2p", bufs=4))
        x1rp = p2b.enter_context(tc.tile_pool(name="x1rp", bufs=2))
        op_ = p2b.enter_context(tc.tile_pool(name="op", bufs=6))
        pmmg = p2b.enter_context(tc.tile_pool(name="pmmg", bufs=4, space="PSUM"))
        pmms = p2b.enter_context(tc.tile_pool(name="pmms", bufs=3, space="PSUM"))

        gv_t = [[None] * NGT for _ in range(n_tiles)]
        x1r_t = [None] * n_tiles

        def stage_grow(t):
            # prefetch x1 reload for the shrink residual
            x1r = x1rp.tile([NP, W], BF16, name=f"x1r_{t}", tag="x1r")
            nc.gpsimd.dma_start(x1r[:], wide_in(x1_d, t * L, L))
            x1r_t[t] = x1r
            for hm in range(NGT):
                psg = pmmg.tile([NP, L], F32, name=f"g2_ps_{t}_{hm}", tag="mmg2")
                for k in range(NCT):
                    lhs = wgr_sb[:, k * HID + hm * NP: k * HID + (hm + 1) * NP]
                    nc.tensor.matmul(psg[:], lhs, x1n_t[t][k][:],
                                     start=(k == 0), stop=(k == NCT - 1))
                psv = pmmg.tile([NP, L], F32, name=f"v_ps_{t}_{hm}", tag="mmg2")
                for k in range(NCT):
                    lhs = wgr_sb[:, k * HID + H2 + hm * NP:
                                 k * HID + H2 + (hm + 1) * NP]
                    nc.tensor.matmul(psv[:], lhs, x1n_t[t][k][:],
                                     start=(k == 0), stop=(k == NCT - 1))
                t2 = t2p.tile([NP, L], BF16, name=f"t2_{t}_{hm}", tag="t2")
                emit_gelu(t2[:], psg[:], t2p, f"t2_{t}_{hm}")
                gvi = gvp.tile([NP, L], BF16, name=f"gv_{t}_{hm}", tag="gv")
                nc.vector.tensor_tensor(gvi[:], t2[:], psv[:], OP.mult)
                gv_t[t][hm] = gvi

        def stage_shrink(t):
            for m in range(NCT):
                ps = pmms.tile([NP, L], F32, name=f"s_ps_{t}_{m}", tag="mms")
                for k in range(2 * NCT):
                    lhs = wsh_sb[:, k * D + m * NP: k * D + (m + 1) * NP]
                    nc.tensor.matmul(ps[:], lhs, gv_t[t][k][:],
                                     start=(k == 0), stop=(k == 2 * NCT - 1))
                om = op_.tile([NP, L], F32, name=f"out_{t}_{m}", tag="out")
                nc.vector.tensor_tensor(om[:], ps[:],
                                        x1r_t[t][:, m * L:(m + 1) * L], OP.add)
                nc.sync.dma_start(
                    out_d.ap()[m * NP:(m + 1) * NP, t * L:(t + 1) * L], om[:])

        stage_grow(0)
        for t in range(1, n_tiles):
            stage_grow(t)
            stage_shrink(t - 1)
        stage_shrink(n_tiles - 1)
        p2b.close()

    nc.compile()
    return nc


def host_prepare(inputs, T_core, n_cores=N_CORES):
    """Build per-core in_maps from full inputs."""
    x = np.asarray(inputs["x"], np.float32)            # [B, T, D]
    B, T, _ = x.shape
    halves = n_cores // B
    assert T == halves * T_core

    gam1 = np.asarray(inputs["hawk_norm_gamma"], np.float32)
    gam2 = np.asarray(inputs["gmlp_norm_gamma"], np.float32)
    scale1 = gam1 * np.sqrt(D)
    scale2 = gam2 * np.sqrt(D)

    wi = (np.asarray(inputs["input_w"], np.float32) * scale1[None, :]).T
    wg = np.asarray(inputs["gates_w"], np.float32).T
    wo = np.asarray(inputs["output_w"], np.float32).T
    wgr = (np.asarray(inputs["grow_w"], np.float32) * scale2[None, :]).T
    wsh = np.asarray(inputs["shrink_w"], np.float32).T

    fb = np.asarray(inputs["forget_base"], np.float64)
    msp = (-8.0 * np.log1p(np.exp(fb))).astype(np.float32)

    def chan_layout(v):  # [D] -> [128, 8] with [p, i] = v[128*i + p]
        return np.ascontiguousarray(v.reshape(NCT, NP).T)

    gb = np.asarray(inputs["gates_b"], np.float32)
    cw = np.asarray(inputs["conv_w"], np.float32)[:, 0, :]   # [D, K]
    cb = np.asarray(inputs["conv_b"], np.float32)

    shared = {
        "wig": np.ascontiguousarray(wi[:, :D]).astype(_BF),
        "wiz": np.ascontiguousarray(wi[:, D:]).astype(_BF),
        "wg": wg.astype(_BF), "wo": wo.astype(_BF),
        "wgr": wgr.astype(_BF), "wsh": wsh.astype(_BF),
        "msp": chan_layout(msp), "msph": chan_layout(0.5 * msp),
        "gbfh": chan_layout(0.5 * gb[:D]), "gbih": chan_layout(0.5 * gb[D:]),
        "cw": np.concatenate([chan_layout(cw[:, k]) for k in range(KCONV)],
                             axis=1),
        "cb": chan_layout(cb),
    }
    in_maps = []
    for core in range(n_cores):
        b, h = core // halves, core % halves
        xf = np.zeros((D, 3 + T_core), np.float32)
        xf[:, 3:] = x[b, h * T_core:(h + 1) * T_core, :].T
        if h > 0:
            xf[:, 0:3] = x[b, h * T_core - 3:h * T_core, :].T
        m = dict(shared)
        m["x"] = xf.astype(_BF)
        m["cmask"] = np.full((NP, 1), 1.0 if h > 0 else 0.0, np.float32)
        in_maps.append(m)
    return in_maps


def assemble_output(results, B, T, T_core, n_cores=N_CORES):
    halves = n_cores // B
    out = np.empty((B, T, D), np.float32)
    for core in range(n_cores):
        b, h = core // halves, core % halves
        out[b, h * T_core:(h + 1) * T_core, :] = results[core]["out"].T
    return out


_PROG_CACHE = {}


def kernel(**inputs) -> np.ndarray:
    x = np.asarray(inputs["x"])
    B, T, _ = x.shape
    T_core = T * B // N_CORES
    L = 512 if T_core % 512 == 0 else T_core // 4
    key = (T_core, L)
    if key not in _PROG_CACHE:
        _PROG_CACHE[key] = build_program(T_core, L)
    nc = _PROG_CACHE[key]
    in_maps = host_prepare(inputs, T_core)
    res = run_bass_kernel_spmd(nc, in_maps, list(range(N_CORES)))
    return assemble_output(res.results, B, T, T_core)


# revision 37
# speedup vs baseline: 1.0325x; 1.0049x over previous
"""Griffin block (Hawk RG-LRU + GatedMLP) Trainium2 Bass kernel, v4.

Sharding: 8 chunks = 4 batches x 2 time-halves, one per NeuronCore.
Per-core layout is feature-major ([channels, tokens]); everything bf16
except the recurrence coefficients (alpha stays f32) and the final
residual adds (psum f32 + bf16 carrier).

v4 notes:
  - sigmoids replaced with tanh (sigma(x) = (1+tanh(x/2))/2 folded into
    downstream scales/biases) so the forget/input gates, alpha and a2
    exps all live in ONE activation-table set (exp_and_others has both
    exp and tanh); Ln resolves to natural_log_exp_and_others via the
    table patch in this file, so the beta chain stays resident too.
  - per-(tensor, tile) wide SBUF tiles with a single batched DMA each
    (a dma_start costs ~0.6us of engine time; v3 issued ~450 of them).
  - phase 1 runs a two-tile software pipeline; hawk gate projection,
    gelu and the carry AllGather fill the scan tail (transition);
    phase 2a (output proj + residual + norm2) then 2b (grow/shrink).
"""

import numpy as np
import ml_dtypes
from contextlib import ExitStack

import concourse.bass as bass
import concourse.bacc as bacc
import concourse.tile as tile
from concourse import mybir
from concourse.bass_utils import run_bass_kernel_spmd

# The act-table-load pass maps each activation to the FIRST table set that
# contains it: Ln -> "natural_log" (ln only), Exp -> "exp_and_others", which
# thrashes a table load on every Ln<->Exp alternation.  Dropping `ln` from
# the ln-only set makes Ln resolve to "natural_log_exp_and_others" (has BOTH
# ln and exp), so ln/exp stretches share one resident set.  Set ids keep
# their act_info.json positions; the hardware tables are unchanged.
import concourse.hw_specs as _hw_specs


def _patched_act_tables(arch):
    tabs = _hw_specs.get_activation_tables(arch)
    out = {}
    for name, fns in tabs.items():
        if name == "natural_log":
            fns = fns - {mybir.ActivationFunctionType.Ln}
        out[name] = fns
    return out


bacc.get_activation_tables = _patched_act_tables
import concourse.bass_interp as _bass_interp
_bass_interp.get_activation_tables = _patched_act_tables

F32 = mybir.dt.float32
BF16 = mybir.dt.bfloat16
AF = mybir.ActivationFunctionType
OP = mybir.AluOpType

D = 1024
NP = 128          # partitions
NCT = D // NP     # channel tiles = 8
KCONV = 4
N_CORES = 8

_BF = ml_dtypes.bfloat16


def build_program(T_core: int, L: int, gelu_approx: bool = False):
    """Emit the SPMD program. T_core tokens per core, token tile L."""
    assert T_core % L == 0
    n_tiles = T_core // L
    H2 = 2 * D        # hawk proj width (2048)
    HID = 2 * H2      # gmlp hidden rows (4096): gate2 [0:2048), v [2048:4096)
    NGT = H2 // NP    # 16 gate/v ctiles
    W = NCT * L       # wide tile free size

    nc = bacc.Bacc("TRN2", target_bir_lowering=False, debug=False,
                   num_devices=N_CORES)

    # ---- DRAM parameters (per-core data via in_maps) ----
    x_d = nc.dram_tensor("x", [D, 3 + T_core], BF16, kind="ExternalInput")
    wiz_d = nc.dram_tensor("wiz", [D, D], BF16, kind="ExternalInput")    # z rows of input_w.T (gamma folded)
    wig_d = nc.dram_tensor("wig", [D, D], BF16, kind="ExternalInput")    # gate rows
    wg_d = nc.dram_tensor("wg", [D, H2], BF16, kind="ExternalInput")     # gates_w.T
    wo_d = nc.dram_tensor("wo", [D, D], BF16, kind="ExternalInput")      # output_w.T
    wgr_d = nc.dram_tensor("wgr", [D, HID], BF16, kind="ExternalInput")  # grow_w.T (gamma folded)
    wsh_d = nc.dram_tensor("wsh", [H2, D], BF16, kind="ExternalInput")   # shrink_w.T
    # per-channel params, laid out [partition, ch_tile]
    msp_d = nc.dram_tensor("msp", [NP, NCT], F32, kind="ExternalInput")    # -8*softplus(fb)
    msph_d = nc.dram_tensor("msph", [NP, NCT], F32, kind="ExternalInput")  # msp/2
    gbfh_d = nc.dram_tensor("gbfh", [NP, NCT], F32, kind="ExternalInput")  # gates_b[:D]/2
    gbih_d = nc.dram_tensor("gbih", [NP, NCT], F32, kind="ExternalInput")  # gates_b[D:]/2
    cw_d = nc.dram_tensor("cw", [NP, KCONV * NCT], F32, kind="ExternalInput")  # conv w taps
    cb_d = nc.dram_tensor("cb", [NP, NCT], F32, kind="ExternalInput")      # conv bias
    cmask_d = nc.dram_tensor("cmask", [NP, 1], F32, kind="ExternalInput")  # 1.0 iff second half

    out_d = nc.dram_tensor("out", [D, T_core], F32, kind="ExternalOutput")

    # ---- internal DRAM scratch ----
    h_d = nc.dram_tensor("h_spill", [D, T_core], BF16)
    ac_d = nc.dram_tensor("ac_spill", [D, T_core], BF16)
    x1_d = nc.dram_tensor("x1_spill", [D, T_core], BF16)
    g_d = nc.dram_tensor("g_spill", [D, T_core], BF16)
    carry_loc = nc.dram_tensor("carry_loc", [1, D], F32)
    carry_gth = nc.dram_tensor("carry_gth", [2, D], F32)

    def wide_in(dram, c0, w):
        """[D', c0:c0+w] -> [128, (D'/128)*w] AP (ctile-major free dim)."""
        return dram.ap()[:, c0:c0 + w].rearrange("(c p) t -> p c t", p=NP)

    with tile.TileContext(nc) as tc, ExitStack() as top:
        # ------- persistent small constants -------
        cpool = top.enter_context(tc.tile_pool(name="consts", bufs=1))
        ones_bf = cpool.tile([NP, NP], BF16, name="ones_bf")
        nc.vector.memset(ones_bf[:], 1.0)
        ones_f = cpool.tile([NP, L], F32, name="ones_f")
        nc.vector.memset(ones_f[:], 1.0)
        msp_sb = cpool.tile([NP, NCT], F32, name="msp_sb")
        nc.sync.dma_start(msp_sb[:], msp_d.ap()[:, :])
        msph_sb = cpool.tile([NP, NCT], F32, name="msph_sb")
        nc.sync.dma_start(msph_sb[:], msph_d.ap()[:, :])
        gbfh_sb = cpool.tile([NP, NCT], F32, name="gbfh_sb")
        nc.sync.dma_start(gbfh_sb[:], gbfh_d.ap()[:, :])
        gbih_sb = cpool.tile([NP, NCT], F32, name="gbih_sb")
        nc.sync.dma_start(gbih_sb[:], gbih_d.ap()[:, :])
        cw_sb = cpool.tile([NP, KCONV * NCT], F32, name="cw_sb")
        nc.sync.dma_start(cw_sb[:], cw_d.ap()[:, :])
        cb_sb = cpool.tile([NP, NCT], F32, name="cb_sb")
        nc.sync.dma_start(cb_sb[:], cb_d.ap()[:, :])
        cmask_sb = cpool.tile([NP, 1], F32, name="cmask_sb")
        nc.sync.dma_start(cmask_sb[:], cmask_d.ap()[:, :])
        hlast = cpool.tile([NP, NCT], F32, name="hlast")
        alast = cpool.tile([NP, NCT], F32, name="alast")
        epsb = cpool.tile([NP, 1], F32, name="epsb")
        nc.vector.memset(epsb[:], 1e-20)
        onepb = cpool.tile([NP, 1], F32, name="onepb")
        nc.vector.memset(onepb[:], 1.0 + 1e-6)
        lhalfb = cpool.tile([NP, 1], F32, name="lhalfb")
        nc.vector.memset(lhalfb[:], float(np.log(0.5)))

        def emit_gelu(out_ap, ps, pool, tag):
            """gelu(ps) -> out; sim lacks Gelu so approx mode builds it."""
            if gelu_approx:
                sg = pool.tile([NP, ps.shape[-1]], F32, name=f"sg_{tag}",
                               tag="gelu_sg")
                nc.scalar.activation(sg[:], ps, AF.Sigmoid, scale=1.702)
                nc.vector.tensor_tensor(out_ap, ps, sg[:], OP.mult)
            else:
                nc.scalar.activation(out_ap, ps, AF.Gelu)

        # xn persists through the transition (gate projs consume it)
        xn_scope = ExitStack()
        xnp = xn_scope.enter_context(
            tc.tile_pool(name="xnp", bufs=NCT * n_tiles))
        xn_t = [None] * n_tiles          # [t][i] -> [NP, L] bf16

        # =========================== PHASE 1 ===========================
        p1 = ExitStack()
        wpool = p1.enter_context(tc.tile_pool(name="w1", bufs=1))
        wiz_sb = wpool.tile([NP, NCT * D], BF16, name="wiz_sb")
        wg_sb = wpool.tile([NP, NCT * H2], BF16, name="wg_sb")

        xp = p1.enter_context(tc.tile_pool(name="xp", bufs=3))
        hxp = p1.enter_context(tc.tile_pool(name="hxp", bufs=NCT + 1))
        sbp = p1.enter_context(tc.tile_pool(name="sbp", bufs=3))   # bf16 temps
        sfp32 = p1.enter_context(tc.tile_pool(name="sfp32", bufs=2))  # f32 temps
        zp = p1.enter_context(tc.tile_pool(name="zp", bufs=NCT + 2))
        zc0p = p1.enter_context(tc.tile_pool(name="zc0p", bufs=4))
        zcbp = p1.enter_context(tc.tile_pool(name="zcbp", bufs=3 * NCT))
        sfp = p1.enter_context(tc.tile_pool(name="sfp", bufs=4))
        ap_ = p1.enter_context(tc.tile_pool(name="ap", bufs=3))
        bp_ = p1.enter_context(tc.tile_pool(name="bp", bufs=2))    # f32 a2/lu2
        bp16 = p1.enter_context(tc.tile_pool(name="bp16", bufs=3))  # bf16 b/sz/xs
        hp = p1.enter_context(tc.tile_pool(name="hp", bufs=1))      # wide h/ac
        zhp = p1.enter_context(tc.tile_pool(name="zhp", bufs=2))
        pmm = p1.enter_context(tc.tile_pool(name="pmm", bufs=5, space="PSUM"))
        pssq = p1.enter_context(tc.tile_pool(name="pssq", bufs=2, space="PSUM"))

        def load_x(t):
            xt = xp.tile([NP, W], BF16, name=f"x_{t}", tag="x")
            nc.sync.dma_start(xt[:], wide_in(x_d, 3 + t * L, L))
            return xt

        def norm_tiles(x_w, w, tag, pool, xsl):
            """s = exp(-0.5*ln(ssq)) = 1/||x||; xn = x*s (bf16).
            x_w wide tile; xsl(i) -> slice of ctile i."""
            ssq = pssq.tile([NP, w], F32, name=f"ssq_{tag}", tag="ssq")
            for i in range(NCT):
                xsq = sbp.tile([NP, w], BF16, name=f"xsq_{tag}_{i}", tag="xsq")
                nc.scalar.activation(xsq[:], xsl(x_w, i), AF.Square)
                nc.tensor.matmul(ssq[:], ones_bf[:], xsq[:],
                                 start=(i == 0), stop=(i == NCT - 1))
            lssq = sfp32.tile([NP, w], F32, name=f"lssq_{tag}", tag="lssq")
            nc.scalar.activation(lssq[:], ssq[:], AF.Ln, bias=epsb[:, 0:1])
            s = sbp.tile([NP, w], BF16, name=f"s_{tag}", tag="s")
            nc.scalar.activation(s[:], lssq[:], AF.Exp, scale=-0.5)
            xn = []
            for i in range(NCT):
                t_ = pool.tile([NP, w], BF16, name=f"xn_{tag}_{i}", tag="xn")
                nc.vector.tensor_tensor(t_[:], xsl(x_w, i), s[:], OP.mult)
                xn.append(t_)
            return xn

        def xsl_L(x_w, i):
            return x_w[:, i * L:(i + 1) * L]

        # ---- halo z: conv inputs for the 3 tokens before this chunk ----
        xh = xp.tile([NP, 3 * NCT], BF16, name="xh", tag="xh")
        nc.sync.dma_start(xh[:], wide_in(x_d, 0, 3))
        nc.sync.dma_start(wiz_sb[:], wide_in(wiz_d, 0, D))
        nc.scalar.dma_start(wg_sb[:], wide_in(wg_d, 0, H2))
        xnh = norm_tiles(xh, 3, "h", hxp, lambda w_, i: w_[:, 3 * i:3 * i + 3])
        zhalo_prev = zhp.tile([NP, 3 * NCT], BF16, name="zhalo_h", tag="zhalo")
        for m in range(NCT):
            ps = pmm.tile([NP, 3], F32, name=f"zh_ps_{m}", tag="mm")
            for k in range(NCT):
                lhs = wiz_sb[:, k * D + m * NP: k * D + (m + 1) * NP]
                nc.tensor.matmul(ps[:], lhs, xnh[k][:],
                                 start=(k == 0), stop=(k == NCT - 1))
            nc.vector.tensor_copy(zhalo_prev[:, 3 * m:3 * m + 3], ps[:])

        x_t = [None] * n_tiles
        zcb_t = [None] * n_tiles

        def stage_AB(t):
            """norm + z-proj + conv for tile t."""
            nonlocal zhalo_prev
            xn_t[t] = norm_tiles(x_t[t], L, f"t{t}", xnp, xsl_L)
            zhalo_cur = zhp.tile([NP, 3 * NCT], BF16, name=f"zhalo_{t}",
                                 tag="zhalo")
            zcb = []
            for m in range(NCT):
                ps = pmm.tile([NP, L], F32, name=f"z_ps_{t}_{m}", tag="mm")
                for k in range(NCT):
                    lhs = wiz_sb[:, k * D + m * NP: k * D + (m + 1) * NP]
                    nc.tensor.matmul(ps[:], lhs, xn_t[t][k][:],
                                     start=(k == 0), stop=(k == NCT - 1))
                zt = zp.tile([NP, L + 3], BF16, name=f"z_{t}_{m}", tag="z")
                nc.gpsimd.tensor_copy(zt[:, 0:3], zhalo_prev[:, 3 * m:3 * m + 3])
                nc.vector.tensor_copy(zt[:, 3:3 + L], ps[:])
                nc.gpsimd.tensor_copy(zhalo_cur[:, 3 * m:3 * m + 3],
                                      zt[:, L:L + 3])
                # depthwise causal conv: tap0 via tensor_scalar (w + bias),
                # taps 1-3 via STT; all bf16
                zc0 = zc0p.tile([NP, L], BF16, name=f"zc_{t}_{m}", tag="zc")
                nc.vector.tensor_scalar(zc0[:], zt[:, 0:L],
                                        cw_sb[:, 0 * NCT + m:0 * NCT + m + 1],
                                        cb_sb[:, m:m + 1],
                                        op0=OP.mult, op1=OP.add)
                for k in (1, 2):
                    nc.vector.scalar_tensor_tensor(
                        zc0[:], zt[:, k:k + L],
                        cw_sb[:, k * NCT + m:k * NCT + m + 1],
                        zc0[:], op0=OP.mult, op1=OP.add)
                zcbm = zcbp.tile([NP, L], BF16, name=f"zcb_{t}_{m}", tag="zcb")
                nc.vector.scalar_tensor_tensor(
                    zcbm[:], zt[:, 3:3 + L],
                    cw_sb[:, 3 * NCT + m:3 * NCT + m + 1],
                    zc0[:], op0=OP.mult, op1=OP.add)
                zcb.append(zcbm)
            zhalo_prev = zhalo_cur
            zcb_t[t] = zcb

        def stage_C(t):
            """gates proj + tanh gates + alpha/beta + scans + spill, tile t.

            sigma(v) = (1+tanh(v/2))/2:
              tf = tanh(0.5*psf + gbf/2)      alpha = exp(msph*tf + msph)
              a2 = exp(msp*tf + msp)          beta' = exp(0.5*lu2 + ln 0.5)
              ti = tanh(0.5*psi + gbi/2)      xs = ((1+ti)*zcb) * beta'
            Scalar blocks are batched per table set (tanh/exp share one)."""
            zcb = zcb_t[t]
            tf = [None] * NCT
            ti = [None] * NCT
            for i in range(NCT):
                psf = pmm.tile([NP, L], F32, name=f"f_ps_{t}_{i}", tag="mm")
                for k in range(NCT):
                    lhs = wg_sb[:, k * H2 + i * NP: k * H2 + (i + 1) * NP]
                    nc.tensor.matmul(psf[:], lhs, zcb[k][:],
                                     start=(k == 0), stop=(k == NCT - 1))
                tfi = sfp.tile([NP, L], BF16, name=f"tf_{t}_{i}", tag="tf")
                nc.scalar.activation(tfi[:], psf[:], AF.Tanh, scale=0.5,
                                     bias=gbfh_sb[:, i:i + 1])
                tf[i] = tfi
                psi = pmm.tile([NP, L], F32, name=f"i_ps_{t}_{i}", tag="mm")
                for k in range(NCT):
                    lhs = wg_sb[:, k * H2 + D + i * NP: k * H2 + D + (i + 1) * NP]
                    nc.tensor.matmul(psi[:], lhs, zcb[k][:],
                                     start=(k == 0), stop=(k == NCT - 1))
                tii = sfp.tile([NP, L], BF16, name=f"ti_{t}_{i}", tag="ti")
                nc.scalar.activation(tii[:], psi[:], AF.Tanh, scale=0.5,
                                     bias=gbih_sb[:, i:i + 1])
                ti[i] = tii

            alpha = [None] * NCT
            a2 = [None] * NCT
            for i in range(NCT):
                al = ap_.tile([NP, L], F32, name=f"al_{t}_{i}", tag="alpha")
                nc.scalar.activation(al[:], tf[i][:], AF.Exp,
                                     scale=msph_sb[:, i:i + 1],
                                     bias=msph_sb[:, i:i + 1])
                alpha[i] = al
                a2i = bp_.tile([NP, L], F32, name=f"a2_{t}_{i}", tag="a2")
                nc.scalar.activation(a2i[:], tf[i][:], AF.Exp,
                                     scale=msp_sb[:, i:i + 1],
                                     bias=msp_sb[:, i:i + 1])
                a2[i] = a2i
            beta = [None] * NCT
            for i in range(NCT):
                lu2 = bp_.tile([NP, L], F32, name=f"lu2_{t}_{i}", tag="lu2")
                nc.scalar.activation(lu2[:], a2[i][:], AF.Ln, scale=-1.0,
                                     bias=onepb[:, 0:1])
                be = bp16.tile([NP, L], BF16, name=f"be_{t}_{i}", tag="beta")
                nc.scalar.activation(be[:], lu2[:], AF.Exp, scale=0.5,
                                     bias=lhalfb[:, 0:1])
                beta[i] = be

            h_w = hp.tile([NP, W], BF16, name=f"h_{t}", tag="h")
            ac_w = hp.tile([NP, W], BF16, name=f"ac_{t}", tag="ac")
            for i in range(NCT):
                sz = bp16.tile([NP, L], BF16, name=f"sz_{t}_{i}", tag="sz")
                nc.vector.scalar_tensor_tensor(sz[:], ti[i][:], 1.0,
                                               zcb[i][:],
                                               op0=OP.add, op1=OP.mult)
                xs = bp16.tile([NP, L], BF16, name=f"xs_{t}_{i}", tag="xs")
                nc.gpsimd.tensor_tensor(xs[:], sz[:], beta[i][:], OP.mult)

                hsl = h_w[:, i * L:(i + 1) * L]
                h_init = 0.0 if t == 0 else hlast[:, i:i + 1]
                nc.vector.tensor_tensor_scan(hsl, alpha[i][:], xs[:], h_init,
                                             op0=OP.mult, op1=OP.add)
                nc.gpsimd.tensor_copy(hlast[:, i:i + 1],
                                      h_w[:, (i + 1) * L - 1:(i + 1) * L])
                asl = ac_w[:, i * L:(i + 1) * L]
                a_init = 1.0 if t == 0 else alast[:, i:i + 1]
                nc.vector.tensor_tensor_scan(asl, alpha[i][:], ones_f[:, 0:L],
                                             a_init, op0=OP.mult, op1=OP.mult)
                nc.gpsimd.tensor_copy(alast[:, i:i + 1],
                                      ac_w[:, (i + 1) * L - 1:(i + 1) * L])
            nc.sync.dma_start(wide_in(h_d, t * L, L), h_w[:])
            nc.gpsimd.dma_start(wide_in(ac_d, t * L, L), ac_w[:])
            if t == n_tiles - 1:
                nc.scalar.dma_start(
                    carry_loc.ap()[0:1, :].rearrange("a (c p) -> p (a c)",
                                                     p=NP),
                    hlast[:])

        # -------- pipelined emission: A/B two tiles ahead of C --------
        x_t[0] = load_x(0)
        if n_tiles > 1:
            x_t[1] = load_x(1)
        stage_AB(0)
        if n_tiles > 1:
            stage_AB(1)
        for t in range(n_tiles):
            if t + 2 < n_tiles:
                x_t[t + 2] = load_x(t + 2)
                stage_AB(t + 2)
            stage_C(t)
            x_t[t] = None  # allow pool reuse

        p1.close()

        # ================= TRANSITION: gate projs + carry =================
        tr = ExitStack()
        wpool_g = tr.enter_context(tc.tile_pool(name="wg2", bufs=1))
        wig_sb = wpool_g.tile([NP, NCT * D], BF16, name="wig_sb")
        nc.sync.dma_start(wig_sb[:], wide_in(wig_d, 0, D))
        pmg = tr.enter_context(tc.tile_pool(name="pmg", bufs=6, space="PSUM"))
        gpool = tr.enter_context(tc.tile_pool(name="gpool", bufs=2))

        # pairwise carry exchange first: gate projs fill the PE while the
        # collective runs; carry reads ride the vector queue so the sync
        # queue's phase-2a reload DMAs are not head-of-line blocked.
        nc.gpsimd.collective_compute(
            "AllGather", OP.bypass,
            replica_groups=[[0, 1], [2, 3], [4, 5], [6, 7]],
            ins=[carry_loc.ap()], outs=[carry_gth.ap()])
        cg = cpool.tile([NP, NCT], F32, name="cg")
        nc.scalar.dma_start(
            cg[:],
            carry_gth.ap()[0:1, :].rearrange("a (c p) -> p (a c)", p=NP))
        carrym = cpool.tile([NP, NCT], F32, name="carrym")
        nc.vector.tensor_scalar(carrym[:], cg[:], cmask_sb[:, 0:1], None,
                                op0=OP.mult)

        for t in range(n_tiles):
            g_w = gpool.tile([NP, W], BF16, name=f"g_{t}", tag="g")
            for m in range(NCT):
                ps = pmg.tile([NP, L], F32, name=f"g_ps_{t}_{m}", tag="mmg")
                for k in range(NCT):
                    lhs = wig_sb[:, k * D + m * NP: k * D + (m + 1) * NP]
                    nc.tensor.matmul(ps[:], lhs, xn_t[t][k][:],
                                     start=(k == 0), stop=(k == NCT - 1))
                emit_gelu(g_w[:, m * L:(m + 1) * L], ps[:], gpool, f"g_{t}_{m}")
            nc.sync.dma_start(wide_in(g_d, t * L, L), g_w[:])
        tr.close()
        xn_scope.close()

        # =========================== PHASE 2a ===========================
        # long-lived pools first (outlive p2a; LIFO discipline)
        wpool_gr = top.enter_context(tc.tile_pool(name="wgr2", bufs=1))
        wgr_sb = wpool_gr.tile([NP, NCT * HID], BF16, name="wgr_sb")
        x1np = top.enter_context(tc.tile_pool(name="x1np",
                                              bufs=NCT * n_tiles))
        p2a = ExitStack()
        wpool_o = p2a.enter_context(tc.tile_pool(name="wo2", bufs=1))
        wo_sb = wpool_o.tile([NP, NCT * D], BF16, name="wo_sb")
        nc.sync.dma_start(wo_sb[:], wide_in(wo_d, 0, D))
        # grow weights start loading now (consumed in 2b)
        nc.scalar.dma_start(wgr_sb[:], wide_in(wgr_d, 0, HID))

        hp2 = p2a.enter_context(tc.tile_pool(name="hp2", bufs=1))
        grp = p2a.enter_context(tc.tile_pool(name="grp", bufs=1))
        xp2 = p2a.enter_context(tc.tile_pool(name="xp2", bufs=1))
        sp2b = p2a.enter_context(tc.tile_pool(name="sp2b", bufs=4))   # bf16
        sp2f = p2a.enter_context(tc.tile_pool(name="sp2f", bufs=1))   # f32
        ghp = p2a.enter_context(tc.tile_pool(name="ghp", bufs=NCT + 2))
        x1bp = p2a.enter_context(tc.tile_pool(name="x1bp", bufs=2))
        pmm2 = p2a.enter_context(tc.tile_pool(name="pmm2", bufs=5, space="PSUM"))
        pssq2 = p2a.enter_context(tc.tile_pool(name="pssq2", bufs=2,
                                               space="PSUM"))
        x1n_t = [[None] * NCT for _ in range(n_tiles)]

        for t in range(n_tiles):
            hr_w = hp2.tile([NP, W], BF16, name=f"hr_{t}", tag="hr")
            nc.sync.dma_start(hr_w[:], wide_in(h_d, t * L, L))
            ac_w = hp2.tile([NP, W], BF16, name=f"acr_{t}", tag="acr")
            nc.sync.dma_start(ac_w[:], wide_in(ac_d, t * L, L))
            gr_w = grp.tile([NP, W], BF16, name=f"grl_{t}", tag="grl")
            nc.sync.dma_start(gr_w[:], wide_in(g_d, t * L, L))
            x2_w = xp2.tile([NP, W], BF16, name=f"x2_{t}", tag="x2")
            nc.sync.dma_start(x2_w[:], wide_in(x_d, 3 + t * L, L))

            gh = []
            for i in range(NCT):
                hf = sp2b.tile([NP, L], BF16, name=f"hf_{t}_{i}", tag="hf")
                nc.vector.scalar_tensor_tensor(
                    hf[:], ac_w[:, i * L:(i + 1) * L], carrym[:, i:i + 1],
                    hr_w[:, i * L:(i + 1) * L], op0=OP.mult, op1=OP.add)
                ghi = ghp.tile([NP, L], BF16, name=f"gh_{t}_{i}", tag="gh")
                nc.vector.tensor_tensor(ghi[:], gr_w[:, i * L:(i + 1) * L],
                                        hf[:], OP.mult)
                gh.append(ghi)

            # output proj + residual -> x1 (bf16, wide) -> spill to DRAM
            x1_w = x1bp.tile([NP, W], BF16, name=f"x1_{t}", tag="x1")
            for m in range(NCT):
                ps = pmm2.tile([NP, L], F32, name=f"o_ps_{t}_{m}", tag="mm2")
                for k in range(NCT):
                    lhs = wo_sb[:, k * D + m * NP: k * D + (m + 1) * NP]
                    nc.tensor.matmul(ps[:], lhs, gh[k][:],
                                     start=(k == 0), stop=(k == NCT - 1))
                nc.vector.tensor_tensor(x1_w[:, m * L:(m + 1) * L], ps[:],
                                        x2_w[:, m * L:(m + 1) * L], OP.add)
            nc.sync.dma_start(wide_in(x1_d, t * L, L), x1_w[:])

            # rmsnorm2 -> x1n (bf16); squares on scalar (in-set everywhere)
            ssq = pssq2.tile([NP, L], F32, name=f"ssq2_{t}", tag="ssq2")
            for i in range(NCT):
                xsq = sp2b.tile([NP, L], BF16, name=f"x1sq_{t}_{i}", tag="x1sq")
                nc.scalar.activation(xsq[:], x1_w[:, i * L:(i + 1) * L],
                                     AF.Square)
                nc.tensor.matmul(ssq[:], ones_bf[:], xsq[:],
                                 start=(i == 0), stop=(i == NCT - 1))
            lssq = sp2f.tile([NP, L], F32, name=f"lssq2_{t}", tag="lssq2")
            nc.scalar.activation(lssq[:], ssq[:], AF.Ln, bias=epsb[:, 0:1])
            s2 = sp2b.tile([NP, L], BF16, name=f"s2_{t}", tag="s2")
            nc.scalar.activation(s2[:], lssq[:], AF.Exp, scale=-0.5)
            for i in range(NCT):
                xni = x1np.tile([NP, L], BF16, name=f"x1n_{t}_{i}", tag="x1n")
                nc.vector.tensor_tensor(xni[:], x1_w[:, i * L:(i + 1) * L],
                                        s2[:], OP.mult)
                x1n_t[t][i] = xni

        p2a.close()

        # =========================== PHASE 2b ===========================
        p2b = ExitStack()
        wpool_s = p2b.enter_context(tc.tile_pool(name="ws2", bufs=1))
        wsh_sb = wpool_s.tile([NP, 2 * NCT * D], BF16, name="wsh_sb")
        nc.sync.dma_start(wsh_sb[:], wide_in(wsh_d, 0, D))

        gvp = p2b.enter_context(tc.tile_pool(name="gvp", bufs=2 * NGT + 2))
        t2p = p2b.enter_context(tc.tile_pool(name="t2p", bufs=4))
        x1rp = p2b.enter_context(tc.tile_pool(name="x1rp", bufs=2))
        op_ = p2b.enter_context(tc.tile_pool(name="op", bufs=6))
        pmmg = p2b.enter_context(tc.tile_pool(name="pmmg", bufs=4, space="PSUM"))
        pmms = p2b.enter_context(tc.tile_pool(name="pmms", bufs=3, space="PSUM"))

        gv_t = [[None] * NGT for _ in range(n_tiles)]
        x1r_t = [None] * n_tiles

        def stage_grow(t):
            # prefetch x1 reload for the shrink residual
            x1r = x1rp.tile([NP, W], BF16, name=f"x1r_{t}", tag="x1r")
            nc.gpsimd.dma_start(x1r[:], wide_in(x1_d, t * L, L))
            x1r_t[t] = x1r
            for hm in range(NGT):
                psg = pmmg.tile([NP, L], F32, name=f"g2_ps_{t}_{hm}", tag="mmg2")
                for k in range(NCT):
                    lhs = wgr_sb[:, k * HID + hm * NP: k * HID + (hm + 1) * NP]
                    nc.tensor.matmul(psg[:], lhs, x1n_t[t][k][:],
                                     start=(k == 0), stop=(k == NCT - 1))
                psv = pmmg.tile([NP, L], F32, name=f"v_ps_{t}_{hm}", tag="mmg2")
                for k in range(NCT):
                    lhs = wgr_sb[:, k * HID + H2 + hm * NP:
                                 k * HID + H2 + (hm + 1) * NP]
                    nc.tensor.matmul(psv[:], lhs, x1n_t[t][k][:],
                                     start=(k == 0), stop=(k == NCT - 1))
                t2 = t2p.tile([NP, L], BF16, name=f"t2_{t}_{hm}", tag="t2")
                emit_gelu(t2[:], psg[:], t2p, f"t2_{t}_{hm}")
                gvi = gvp.tile([NP, L], BF16, name=f"gv_{t}_{hm}", tag="gv")
                nc.vector.tensor_tensor(gvi[:], t2[:], psv[:], OP.mult)
                gv_t[t][hm] = gvi

        def stage_shrink(t):
            for m in range(NCT):
                ps = pmms.tile([NP, L], F32, name=f"s_ps_{t}_{m}", tag="mms")
                for k in range(2 * NCT):
                    lhs = wsh_sb[:, k * D + m * NP: k * D + (m + 1) * NP]
                    nc.tensor.matmul(ps[:], lhs, gv_t[t][k][:],
                                     start=(k == 0), stop=(k == 2 * NCT - 1))
                om = op_.tile([NP, L], F32, name=f"out_{t}_{m}", tag="out")
                nc.vector.tensor_tensor(om[:], ps[:],
                                        x1r_t[t][:, m * L:(m + 1) * L], OP.add)
                nc.sync.dma_start(
                    out_d.ap()[m * NP:(m + 1) * NP, t * L:(t + 1) * L], om[:])

        stage_grow(0)
        for t in range(1, n_tiles):
            stage_grow(t)
            stage_shrink(t - 1)
        stage_shrink(n_tiles - 1)
        p2b.close()

    nc.compile()
    return nc


def host_prepare(inputs, T_core, n_cores=N_CORES):
    """Build per-core in_maps from full inputs."""
    x = np.asarray(inputs["x"], np.float32)            # [B, T, D]
    B, T, _ = x.shape
    halves = n_cores // B
    assert T == halves * T_core

    gam1 = np.asarray(inputs["hawk_norm_gamma"], np.float32)
    gam2 = np.asarray(inputs["gmlp_norm_gamma"], np.float32)
    scale1 = gam1 * np.sqrt(D)
    scale2 = gam2 * np.sqrt(D)

    wi = (np.asarray(inputs["input_w"], np.float32) * scale1[None, :]).T
    wg = np.asarray(inputs["gates_w"], np.float32).T
    wo = np.asarray(inputs["output_w"], np.float32).T
    wgr = (np.asarray(inputs["grow_w"], np.float32) * scale2[None, :]).T
    wsh = np.asarray(inputs["shrink_w"], np.float32).T

    fb = np.asarray(inputs["forget_base"], np.float64)
    msp = (-8.0 * np.log1p(np.exp(fb))).astype(np.float32)

    def chan_layout(v):  # [D] -> [128, 8] with [p, i] = v[128*i + p]
        return np.ascontiguousarray(v.reshape(NCT, NP).T)

    gb = np.asarray(inputs["gates_b"], np.float32)
    cw = np.asarray(inputs["conv_w"], np.float32)[:, 0, :]   # [D, K]
    cb = np.asarray(inputs["conv_b"], np.float32)

    shared = {
        "wig": np.ascontiguousarray(wi[:, :D]).astype(_BF),
        "wiz": np.ascontiguousarray(wi[:, D:]).astype(_BF),
        "wg": wg.astype(_BF), "wo": wo.astype(_BF),
        "wgr": wgr.astype(_BF), "wsh": wsh.astype(_BF),
        "msp": chan_layout(msp), "msph": chan_layout(0.5 * msp),
        "gbfh": chan_layout(0.5 * gb[:D]), "gbih": chan_layout(0.5 * gb[D:]),
        "cw": np.concatenate([chan_layout(cw[:, k]) for k in range(KCONV)],
                             axis=1),
        "cb": chan_layout(cb),
    }
    in_maps = []
    for core in range(n_cores):
        b, h = core // halves, core % halves
        xf = np.zeros((D, 3 + T_core), np.float32)
        xf[:, 3:] = x[b, h * T_core:(h + 1) * T_core, :].T
        if h > 0:
            xf[:, 0:3] = x[b, h * T_core - 3:h * T_core, :].T
        m = dict(shared)
        m["x"] = xf.astype(_BF)
        m["cmask"] = np.full((NP, 1), 1.0 if h > 0 else 0.0, np.float32)
        in_maps.append(m)
    return in_maps


def assemble_output(results, B, T, T_core, n_cores=N_CORES):
    halves = n_cores // B
    out = np.empty((B, T, D), np.float32)
    for core in range(n_cores):
        b, h = core // halves, core % halves
        out[b, h * T_core:(h + 1) * T_core, :] = results[core]["out"].T
    return out


_PROG_CACHE = {}


def kernel(**inputs) -> np.ndarray:
    x = np.asarray(inputs["x"])
    B, T, _ = x.shape
    T_core = T * B // N_CORES
    L = 512 if T_core % 512 == 0 else T_core // 4
    key = (T_core, L)
    if key not in _PROG_CACHE:
        _PROG_CACHE[key] = build_program(T_core, L)
    nc = _PROG_CACHE[key]
    in_maps = host_prepare(inputs, T_core)
    res = run_bass_kernel_spmd(nc, in_maps, list(range(N_CORES)))
    return assemble_output(res.results, B, T, T_core)
